# revision 14
# baseline (speedup 1.0000x reference)
"""Block-causal GQA attention for Trainium2, 8 NeuronCores.

Sharding: core = (batch b, GQA group g): 2 batches x 4 kv-groups.
Each core computes its 4 q-heads + 1 kv-head on one batch element in a
transposed layout (head_dim on partitions, tokens on free dim), then a
row-parallel partial out-projection; the host sums the 4 partials per batch.

v3 structure (vs v2 baseline):
  - All DRAM inputs host-packed to SBUF layout (contiguous DMAs); critical
    path (wq, x chunk 0) leads the SP queue, tables on the Act queue,
    phase-A-only data on the Pool queue.
  - One PSUM pool set for the whole kernel (tags A1/A2/pv = 8 banks), no
    phase-boundary barrier.
  - Phase P: k-RMS folded into kt2 (pre-rope scale), q/k norm sqrt+recip
    merged into one scalar + one DVE call via tile-position packing into a
    [66,512] tile, rotate-half via strip DMAs (off the DVE), norm-broadcast
    matmuls deferred to a P2 pass so the PE queue never stalls on the
    sqrt/recip chain.
  - Phase A: exp with immediate scale/bias (k-norm pre-applied), PV pair
    merged into one matmul (2D free AP), 2-entry scores lookahead, lazy
    softmax normalization (unnormalized evacuation releases the PV PSUM
    early; normalize off-chain before the out-projection).
"""
import numpy as np
import ml_dtypes

B, S, DIM = 2, 2048, 1024
H, KVH, HD = 16, 4, 64
EPS = 1e-6
SCALE = HD ** -0.5
PT_TILES = S // 128  # 16
N_CHUNK = 512
N_CHUNKS = S // N_CHUNK  # 4

_BUILD_CACHE = {}
_BLOCKIND = np.zeros((34, 128), np.float32)
_BLOCKIND[0, 0:64] = 1.0
_BLOCKIND[1, 64:128] = 1.0
_BLOCKIND[32, 0:64] = 1.0
_BLOCKIND[33, 64:128] = 1.0


def _analyze_mask(mask):
    """Classify 128x128 tiles: 0=skip, 1=full, 2=mixed. Returns status grid,
    mixed tile stack (transposed to (k,q) layout, 0/1 float32), and index map.
    Index 0 of the stack is always the all-zero tile."""
    T = PT_TILES
    status = np.zeros((T, T), np.int8)
    tiles = [np.zeros((128, 128), np.float32)]
    idx = {}
    m = np.asarray(mask)
    for i in range(T):
        for j in range(T):
            sub = m[i * 128:(i + 1) * 128, j * 128:(j + 1) * 128]
            if not sub.any():
                status[i, j] = 0
            elif sub.all():
                status[i, j] = 1
            else:
                status[i, j] = 2
                idx[(i, j)] = len(tiles)
                tiles.append(np.ascontiguousarray(sub.T).astype(np.float32))
    return status, np.stack(tiles), idx


def _make_schedule(status, idx):
    """Per chunk: list of (ktile j, s0, s1, [(subtile s, mask_tile_index)])
    where [s0*128, s1*128) is the contiguous span of alive q-subtiles and the
    list holds per-subtile multiplies (zero tile for dead-in-span, mixed id
    for partial)."""
    sched = []
    for ci in range(N_CHUNKS):
        qts = list(range(4 * ci, 4 * ci + 4))
        entries = []
        for j in range(PT_TILES):
            st = [status[i, j] for i in qts]
            if not any(st):
                continue
            alive = [s for s in range(4) if st[s] != 0]
            s0, s1 = alive[0], alive[-1] + 1
            mults = []
            for s in range(s0, s1):
                if st[s] == 1:
                    continue
                mults.append((s, 0 if st[s] == 0 else idx[(qts[s], j)]))
            entries.append((j, s0, s1, mults))
        sched.append(entries)
    return sched


def _build(sched_key, sched, n_masks, neg_c, debug=False):
    import concourse.bacc as bacc
    import concourse.mybir as mybir
    import concourse.tile as tile
    from concourse.masks import make_identity

    F32 = mybir.dt.float32
    F32R = mybir.dt.float32r
    BF16 = mybir.dt.bfloat16
    AF = mybir.ActivationFunctionType

    nc = bacc.Bacc("TRN2", target_bir_lowering=False, debug=False)
    xtc = nc.dram_tensor("xtc", (N_CHUNKS, 128, 8, N_CHUNK), BF16,
                         kind="ExternalInput").ap()
    wq = nc.dram_tensor("wq", (128, 8, 256), BF16, kind="ExternalInput").ap()
    wkv = nc.dram_tensor("wkv", (128, 8, 128), BF16, kind="ExternalInput").ap()
    wo = nc.dram_tensor("wo", (128, 2, DIM), BF16, kind="ExternalInput").ap()
    cq = nc.dram_tensor("cq", (128, S), BF16, kind="ExternalInput").ap()
    sqp = nc.dram_tensor("sqp", (128, S), BF16, kind="ExternalInput").ap()
    ck = nc.dram_tensor("ck", (64, S), BF16, kind="ExternalInput").ap()
    skp = nc.dram_tensor("skp", (64, S), BF16, kind="ExternalInput").ap()
    masks = nc.dram_tensor("masks", (n_masks, 128, 128), BF16,
                           kind="ExternalInput").ap()
    blockind_d = nc.dram_tensor("blockind", (34, 128), F32,
                                kind="ExternalInput").ap()
    outc = nc.dram_tensor("outc", (8, N_CHUNKS, 128, N_CHUNK), BF16,
                          kind="ExternalOutput").ap()
    if debug:
        d_t1a0 = nc.dram_tensor("d_t1a0", (128, S), BF16, kind="ExternalOutput").ap()
        d_t1a1 = nc.dram_tensor("d_t1a1", (128, S), BF16, kind="ExternalOutput").ap()
        d_kt2 = nc.dram_tensor("d_kt2", (128, S), BF16, kind="ExternalOutput").ap()
        d_vaug = nc.dram_tensor("d_vaug", (128, PT_TILES, 65), BF16, kind="ExternalOutput").ap()
        d_an = nc.dram_tensor("d_an", (128, N_CHUNK), BF16, kind="ExternalOutput").ap()
        d_rd = nc.dram_tensor("d_rd", (1, 2, N_CHUNK), mybir.dt.float32, kind="ExternalOutput").ap()
        d_au = nc.dram_tensor("d_au", (64, 2, N_CHUNK), BF16, kind="ExternalOutput").ap()

    with tile.TileContext(nc) as tc:
        with tc.tile_pool(name="persist", bufs=1) as pp, \
             tc.tile_pool(name="work", bufs=2) as pw, \
             tc.tile_pool(name="ps", bufs=2, space="PSUM") as ps, \
             tc.tile_pool(name="psv", bufs=1, space="PSUM") as psv:

            # --- persistent tiles / initial DMAs --------------------------
            # critical path first on SP queue: wq then x chunk 0
            wq_sb = pp.tile([128, 8, 256], BF16)
            nc.sync.dma_start(out=wq_sb[:, 0:4, :], in_=wq[:, 0:4, :])
            nc.scalar.dma_start(out=wq_sb[:, 4:8, :], in_=wq[:, 4:8, :])
            # Act queue: everything phase P needs soon
            wkv_sb = pp.tile([128, 8, 128], BF16)
            nc.scalar.dma_start(out=wkv_sb, in_=wkv)
            cq_sb = pp.tile([128, S], BF16)
            nc.scalar.dma_start(out=cq_sb, in_=cq)
            sqp_sb = pp.tile([128, S], BF16)
            nc.scalar.dma_start(out=sqp_sb, in_=sqp)
            ck_sb = pp.tile([64, S], BF16)
            nc.scalar.dma_start(out=ck_sb, in_=ck)
            skp_sb = pp.tile([64, S], BF16)
            nc.scalar.dma_start(out=skp_sb, in_=skp)
            # Pool queue: phase-A-only data
            blockind = pp.tile([34, 128], F32)
            nc.gpsimd.dma_start(out=blockind, in_=blockind_d)
            masks_sb = pp.tile([128, n_masks, 128], BF16)
            nc.gpsimd.dma_start(out=masks_sb,
                                in_=masks.rearrange("n k q -> k n q"))
            wo_sb = pp.tile([128, 2, DIM], BF16)
            nc.gpsimd.dma_start(out=wo_sb, in_=wo)

            # persistent compute state
            t1s = pp.tile([128, 2, S], BF16)     # roped q, un-normalized
            t1a = [pp.tile([128, S], BF16, tag=f"t1a{m}", name=f"t1a{m}")
                   for m in range(2)]            # roped+normalized q per m
            kt2 = pp.tile([128, S], BF16)        # normalized roped k (dup)
            vst = pp.tile([64, S], BF16)         # v staging (d on part)
            v_aug = pp.tile([128, PT_TILES, 65], BF16)
            nrq = pp.tile([66, N_CHUNKS, N_CHUNK], F32)   # 1/rms: q m0@0:2, m1@32:34, k@64:65
            rkb = pp.tile([64, N_CHUNKS, N_CHUNK], F32)   # k 1/rms bcast 64 part

            ones1 = pp.tile([128, 1], BF16)
            nc.vector.memset(ones1, 1.0)
            nc.vector.tensor_copy(v_aug[:, :, 64:65],
                                  ones1[:].broadcast_to([128, PT_TILES, 1]))
            onesq = pp.tile([128, 2], BF16)
            nc.vector.memset(onesq, 0.0)
            nc.vector.memset(onesq[0:64, 0:1], 1.0)
            nc.vector.memset(onesq[64:128, 1:2], 1.0)
            onesk1 = pp.tile([64, 1], BF16)
            nc.vector.memset(onesk1, 1.0)
            ident_bf = pp.tile([64, 64], BF16)
            make_identity(nc, ident_bf[:])
            eps66 = pp.tile([66, 1], F32)
            nc.vector.memset(eps66, EPS)
            warm = pp.tile([1, 1], F32)
            nc.scalar.activation(out=warm[:], in_=eps66[0:1, 0:1],
                                 func=AF.Log)

            # ============ interleaved P/A pipeline =======================
            # P work for chunk cj+1 is emitted as "blobs" inside the
            # attention groups of chunk cj; one activation table
            # (natural_log_exp) serves the whole kernel: rsqrt = exp(-.5 ln).
            st8 = {}

            def blob_xt(cj):
                t = px.tile([128, 8, N_CHUNK], BF16, tag="xt", name=f"xt{cj}")
                nc.sync.dma_start(out=t, in_=xtc[cj])
                st8[("xt", cj)] = t

            def blob_q(cj, m):
                xt = st8[("xt", cj)]
                if ("sqq", cj) not in st8:
                    st8[("sqq", cj)] = pw.tile([128, 2, N_CHUNK], BF16,
                                               tag="sqq", name=f"sqq{cj}")
                    st8[("qtr", cj)] = pw.tile([128, 2, N_CHUNK], BF16,
                                               tag="qtr", name=f"qtr{cj}")
                sqq, qtr = st8[("sqq", cj)], st8[("qtr", cj)]
                qp = ps.tile([128, N_CHUNK], F32, tag="A1", name=f"qps{cj}_{m}")
                for k in range(8):
                    nc.tensor.matmul(qp[:], wq_sb[:, k, m * 128:(m + 1) * 128],
                                     xt[:, k, :], start=(k == 0), stop=(k == 7))
                nc.scalar.activation(out=sqq[:, m, :], in_=qp[:],
                                     func=AF.Square)
                nc.scalar.copy(out=qtr[:, m, :], in_=qp[:])

            def blob_kv(cj):
                xt = st8[("xt", cj)]
                off = cj * N_CHUNK
                kv_ps = ps.tile([128, N_CHUNK], F32, tag="A2", name=f"kvps{cj}")
                for k in range(8):
                    nc.tensor.matmul(kv_ps[:], wkv_sb[:, k, :], xt[:, k, :],
                                     start=(k == 0), stop=(k == 7))
                sqk = pw.tile([64, N_CHUNK], BF16, tag="sqk", name=f"sqk{cj}")
                nc.scalar.activation(out=sqk[:], in_=kv_ps[0:64, :],
                                     func=AF.Square)
                ktr_raw = pw.tile([64, N_CHUNK], BF16, tag="ktraw",
                                  name=f"ktraw{cj}")
                nc.vector.tensor_copy(ktr_raw[:], kv_ps[0:64, :])
                nc.vector.tensor_copy(vst[:, off:off + N_CHUNK],
                                      kv_ps[64:128, :])
                st8[("sqk", cj)] = sqk
                st8[("ktraw", cj)] = ktr_raw

            def blob_norms(cj):
                sqq, sqk = st8[("sqq", cj)], st8[("sqk", cj)]
                nrm = ps.tile([66, N_CHUNK], F32, tag="A1", name=f"nrm{cj}")
                nc.tensor.matmul(nrm[0:2, :], onesq[:], sqq[:, 0, :],
                                 start=True, stop=True)
                nc.tensor.matmul(nrm[32:34, :], onesq[:], sqq[:, 1, :],
                                 start=True, stop=True)
                nc.tensor.matmul(nrm[64:65, :], onesk1[:], sqk[:],
                                 start=True, stop=True)
                nsb = pw.tile([66, N_CHUNK], F32, tag="nsb", name=f"nsb{cj}")
                nc.scalar.activation(out=nsb[:], in_=nrm[:], func=AF.Ln,
                                     bias=eps66[:], scale=1.0 / HD)
                nc.scalar.activation(out=nrq[:, cj, :], in_=nsb[:],
                                     func=AF.Exp, scale=-0.5)
                rk0 = pw.tile([1, N_CHUNK], BF16, tag="rk0", name=f"rk0{cj}")
                nc.vector.tensor_copy(rk0[:], nrq[64:65, cj, :])
                rkb = pw.tile([64, N_CHUNK], BF16, tag="rkb", name=f"rkb{cj}")
                nc.gpsimd.partition_broadcast(rkb[:], rk0[:], channels=64)
                st8[("rkb", cj)] = rkb

            def blob_ktail(cj):
                off = cj * N_CHUNK
                ktr_raw, rkb = st8[("ktraw", cj)], st8[("rkb", cj)]
                ktr = pw.tile([64, N_CHUNK], BF16, tag="ktr", name=f"ktr{cj}")
                nc.vector.tensor_mul(ktr[:], ktr_raw[:], rkb[:])
                uk = pw.tile([64, N_CHUNK], BF16, tag="uk", name=f"uk{cj}")
                nc.vector.tensor_mul(uk[:], ktr[:], skp_sb[:, off:off + N_CHUNK])
                tk = pw.tile([64, N_CHUNK], BF16, tag="tk", name=f"tk{cj}")
                nc.vector.tensor_mul(tk[:], ktr[:], ck_sb[:, off:off + N_CHUNK])
                uksw = pw.tile([64, N_CHUNK], BF16, tag="uksw", name=f"uksw{cj}")
                nc.sync.dma_start(out=uksw[0:32, :], in_=uk[32:64, :])
                nc.sync.dma_start(out=uksw[32:64, :], in_=uk[0:32, :])
                # q rope
                qtr = st8[("qtr", cj)]
                uq = pw.tile([128, 2, N_CHUNK], BF16, tag="uq", name=f"uq{cj}")
                tq = pw.tile([128, 2, N_CHUNK], BF16, tag="tq", name=f"tq{cj}")
                for m in range(2):
                    nc.vector.tensor_mul(uq[:, m, :], qtr[:, m, :],
                                         sqp_sb[:, off:off + N_CHUNK])
                    nc.vector.tensor_mul(tq[:, m, :], qtr[:, m, :],
                                         cq_sb[:, off:off + N_CHUNK])
                usw = pw.tile([128, 2, N_CHUNK], BF16, tag="usw",
                              name=f"usw{cj}")
                for blk, srcp in enumerate((32, 0, 96, 64)):
                    nc.sync.dma_start(
                        out=usw[blk * 32:(blk + 1) * 32, :, :],
                        in_=uq[srcp:srcp + 32, :, :])
                nc.vector.tensor_add(kt2[0:64, off:off + N_CHUNK], tk[:],
                                     uksw[:])
                nc.sync.dma_start(out=kt2[64:128, off:off + N_CHUNK],
                                  in_=kt2[0:64, off:off + N_CHUNK])
                nc.vector.tensor_add(t1s[:, :, off:off + N_CHUNK], tq[:],
                                     usw[:])

            def blob_p2(cj):
                off = cj * N_CHUNK
                for m in range(2):
                    o = 32 * m
                    rep = ps.tile([128, N_CHUNK], F32, tag="A1",
                                  name=f"rep{cj}_{m}")
                    nc.tensor.matmul(rep[:], blockind[o:o + 2, :],
                                     nrq[o:o + 2, cj, :],
                                     start=True, stop=True)
                    nc.vector.tensor_mul(t1a[m][:, off:off + N_CHUNK],
                                         t1s[:, m, off:off + N_CHUNK], rep[:])
                for t in range(4):
                    j = 4 * cj + t
                    tr_ps = ps.tile([128, 64], BF16, tag="A1",
                                    name=f"tr{cj}_{t}")
                    nc.tensor.transpose(tr_ps[:],
                                        vst[:, j * 128:(j + 1) * 128],
                                        ident_bf[:])
                    nc.vector.tensor_copy(v_aug[:, j, 0:64], tr_ps[:])

            # ======================= PHASE A =============================
            outproj_pending = []
            attn_done = {}

            def issue_outproj_one():
                ci_, mo = outproj_pending.pop(0)
                a0, a1 = attn_done[ci_]
                o_ps = ps.tile([128, N_CHUNK], F32, tag="A1",
                               name=f"ops{ci_}_{mo}")
                nc.tensor.matmul(o_ps[:], wo_sb[:, 0, mo * 128:(mo + 1) * 128],
                                 a0[:], start=True, stop=False)
                nc.tensor.matmul(o_ps[:], wo_sb[:, 1, mo * 128:(mo + 1) * 128],
                                 a1[:], start=False, stop=True)
                o_sb = pw.tile([128, N_CHUNK], BF16, tag="osb", bufs=3,
                               name=f"osb{ci_}_{mo}")
                nc.vector.tensor_copy(o_sb[:], o_ps[:])
                nc.gpsimd.dma_start(out=outc[mo, ci_], in_=o_sb[:])

            def phase2(m, ci, blobs):
                off = ci * N_CHUNK
                entries = sched[ci]
                n_ent = len(entries)
                # blob k fires after entry floor((k+1)*n/(nb+1))
                fire = {}
                for k in range(len(blobs)):
                    e = min(n_ent - 1, (k + 1) * n_ent // (len(blobs) + 1))
                    fire.setdefault(e, []).append(blobs[k])
                pv = psv.tile([65, 2, N_CHUNK], F32, tag="pv",
                              name=f"pv{m}_{ci}")
                hist = []

                def pv_mm(e_idx):
                    a, b_, j, pt = hist[e_idx]
                    for hh in range(2):
                        nc.tensor.matmul(pv[:, hh, a:b_], v_aug[:, j, :],
                                         pt[:, hh, a:b_],
                                         start=(e_idx == 0),
                                         stop=(e_idx == n_ent - 1))

                for idx_e, (j, s0, s1, mults) in enumerate(entries):
                    koff = j * 128
                    a, b_ = s0 * 128, s1 * 128
                    st = ps.tile([128, 2, N_CHUNK], F32, tag="A2",
                                 name=f"st{m}_{ci}_{j}")
                    nc.tensor.matmul(
                        st[:, 0, a:b_],
                        kt2[0:64, koff:koff + 128],
                        t1a[m][0:64, off + a:off + b_],
                        start=True, stop=True)
                    nc.tensor.matmul(
                        st[:, 1, a:b_],
                        kt2[64:128, koff:koff + 128],
                        t1a[m][64:128, off + a:off + b_],
                        start=True, stop=True, tile_position=(64, 0))
                    pt = pw.tile([128, 2, N_CHUNK], BF16, tag="pt", bufs=4,
                                 name=f"pt{m}_{ci}_{j}")
                    nc.scalar.activation(
                        out=pt[:, :, a:b_], in_=st[:, :, a:b_],
                        func=AF.Exp, bias=neg_c, scale=1.0)
                    for s_, mt in mults:
                        nc.vector.tensor_mul(
                            pt[:, :, s_ * 128:(s_ + 1) * 128],
                            pt[:, :, s_ * 128:(s_ + 1) * 128],
                            masks_sb[:, mt:mt + 1, :].broadcast_to([128, 2, 128]))
                    hist.append((a, b_, j, pt))
                    if idx_e >= 2:
                        pv_mm(idx_e - 2)
                    for fn in fire.get(idx_e, ()):
                        fn()
                    if outproj_pending and 2 * idx_e >= n_ent:
                        issue_outproj_one()
                pv_mm(n_ent - 2)
                pv_mm(n_ent - 1)

                # lazy normalization: evacuate unnormalized (releases pv),
                # normalize off the pv-WAR chain
                au = pw.tile([64, 2, N_CHUNK], BF16, tag="au",
                             name=f"au{m}_{ci}")
                nc.vector.tensor_copy(au[:], pv[0:64, :, :])
                dsb = pw.tile([1, 2, N_CHUNK], F32, tag="dsb",
                              name=f"dsb{m}_{ci}")
                nc.vector.tensor_copy(dsb[:], pv[64:65, :, :])
                rd = pw.tile([1, 2, N_CHUNK], F32, tag="rd",
                             name=f"rd{m}_{ci}")
                nc.vector.reciprocal_approx_fast(out=rd[:], in_=dsb[:])
                rd16 = pw.tile([1, 2, N_CHUNK], BF16, tag="rd16",
                               name=f"rd16{m}_{ci}")
                nc.vector.tensor_copy(rd16[:], rd[:])
                rdb = pw.tile([64, 2, N_CHUNK], BF16, tag="rdb",
                              name=f"rdb{m}_{ci}")
                nc.gpsimd.partition_broadcast(rdb[:], rd16[:], channels=64)
                an = pw.tile([128, N_CHUNK], BF16, tag="an", bufs=4,
                             name=f"an{m}_{ci}")
                for hh in range(2):
                    nc.vector.tensor_mul(
                        an[hh * 64:(hh + 1) * 64, :],
                        au[:, hh, :], rdb[:, hh, :])
                if debug and m == 0 and ci == 0:
                    nc.sync.dma_start(out=d_an, in_=an[:])
                    nc.sync.dma_start(out=d_rd, in_=rd[:])
                    nc.sync.dma_start(out=d_au, in_=au[:])
                return an

            with tc.tile_pool(name="px", bufs=2) as px:
                # chunk 0's P runs up front
                blob_xt(0)
                blob_q(0, 0)
                blob_q(0, 1)
                blob_kv(0)
                blob_norms(0)
                blob_ktail(0)
                blob_p2(0)

                for ci in range(N_CHUNKS):
                    cn = ci + 1
                    if cn < N_CHUNKS:
                        m0b = [lambda c=cn: blob_xt(c),
                               lambda c=cn: blob_q(c, 0),
                               lambda c=cn: blob_q(c, 1),
                               lambda c=cn: blob_kv(c)]
                        m1b = [lambda c=cn: blob_norms(c),
                               lambda c=cn: blob_ktail(c),
                               lambda c=cn: blob_p2(c)]
                    else:
                        m0b, m1b = [], []
                    a0 = phase2(0, ci, m0b)
                    a1 = phase2(1, ci, m1b)
                    attn_done[ci] = (a0, a1)
                    outproj_pending.extend((ci, mo) for mo in range(8))
                while outproj_pending:
                    issue_outproj_one()
                if debug:
                    nc.sync.dma_start(out=d_t1a0, in_=t1a[0][:])
                    nc.sync.dma_start(out=d_t1a1, in_=t1a[1][:])
                    nc.sync.dma_start(out=d_kt2, in_=kt2[:])
                    nc.sync.dma_start(out=d_vaug, in_=v_aug[:])

    nc.compile()
    return nc


def _get_nc(sched_key, sched, n_masks, neg_c):
    key = (sched_key, n_masks, float(neg_c))
    if key not in _BUILD_CACHE:
        _BUILD_CACHE[key] = _build(sched_key, sched, n_masks, neg_c)
    return _BUILD_CACHE[key]


def kernel(x, Wq, Wkv, Wo, q_norm_w, k_norm_w, rope_cos, rope_sin,
           attention_mask):
    x = np.asarray(x, dtype=np.float32)
    Wq = np.asarray(Wq, dtype=np.float32)
    Wkv = np.asarray(Wkv, dtype=np.float32)
    Wo = np.asarray(Wo, dtype=np.float32)
    qw = np.asarray(q_norm_w, dtype=np.float32)
    kw = np.asarray(k_norm_w, dtype=np.float32)
    cos = np.asarray(rope_cos, dtype=np.float32)
    sin = np.asarray(rope_sin, dtype=np.float32)

    status, mask_tiles, idx = _analyze_mask(attention_mask)
    sched = _make_schedule(status, idx)
    sched_key = status.tobytes()

    # numerically safe exp shift (0 in the normal regime)
    mct_q = max(np.abs(cos).max(), np.abs(sin).max(), 1e-9)
    bound = SCALE * 2.0 * HD * mct_q * mct_q \
        * max(np.abs(qw).max(), 1e-9) * max(np.abs(kw).max(), 1e-9)
    neg_c = -max(0.0, float(bound) - 60.0)

    nc = _get_nc(sched_key, sched, mask_tiles.shape[0], neg_c)

    # host-folded rope tables (transposed layout, head-dim on partitions)
    half = HD // 2
    swap = np.concatenate([np.arange(half, HD), np.arange(0, half)])
    sgn = np.concatenate([-np.ones(half, np.float32), np.ones(half, np.float32)])
    BF = ml_dtypes.bfloat16
    # cq[e,t] = qw[e]*SCALE*cos[t,e];  sqp[e,t] = sgn[swap[e]]*qw[e]*SCALE*sin[t,swap[e]]
    cosq_h = (cos.T * (qw * SCALE)[:, None]).astype(np.float32)        # (64,S)
    sinq_h = (sin.T[swap, :] * (sgn[swap] * qw * SCALE)[:, None]).astype(np.float32)
    cosk_h = (cos.T * kw[:, None]).astype(np.float32)
    sink_h = (sin.T[swap, :] * (sgn[swap] * kw)[:, None]).astype(np.float32)
    cq_b = np.ascontiguousarray(np.concatenate([cosq_h, cosq_h], 0)).astype(BF)
    sqp_b = np.ascontiguousarray(np.concatenate([sinq_h, sinq_h], 0)).astype(BF)
    ck_b = np.ascontiguousarray(cosk_h).astype(BF)
    skp_b = np.ascontiguousarray(sink_h).astype(BF)
    masks_b = mask_tiles.astype(BF)

    # x chunks in SBUF layout: (chunk, 128 part, 8 k, 512 n)
    xtc_b = []
    for b in range(B):
        xT = np.ascontiguousarray(x[b].T)                  # (DIM, S)
        v = xT.reshape(8, 128, N_CHUNKS, N_CHUNK).transpose(2, 1, 0, 3)
        xtc_b.append(np.ascontiguousarray(v).astype(BF))

    in_maps = []
    for c in range(8):
        b, g = c // 4, c % 4
        wq_s = Wq[:, g * 256:(g + 1) * 256]                # (1024, 256)
        wq_p = np.ascontiguousarray(
            wq_s.reshape(8, 128, 256).transpose(1, 0, 2)).astype(BF)
        wkv_s = np.concatenate(
            [Wkv[:, g * HD:(g + 1) * HD],
             Wkv[:, KVH * HD + g * HD: KVH * HD + (g + 1) * HD]], axis=1)
        wkv_p = np.ascontiguousarray(
            wkv_s.reshape(8, 128, 128).transpose(1, 0, 2)).astype(BF)
        wo_s = Wo[g * 256:(g + 1) * 256, :]                # (256, 1024)
        wo_p = np.ascontiguousarray(
            wo_s.reshape(2, 128, DIM).transpose(1, 0, 2)).astype(BF)
        im = {
            "xtc": xtc_b[b],
            "wq": wq_p,
            "wkv": wkv_p,
            "wo": wo_p,
            "cq": cq_b, "sqp": sqp_b,
            "ck": ck_b, "skp": skp_b,
            "masks": masks_b,
            "blockind": _BLOCKIND,
        }
        in_maps.append(im)

    from concourse.bass_utils import run_bass_kernel_spmd
    res = run_bass_kernel_spmd(nc, in_maps, core_ids=list(range(8)), trace=False)

    out = np.zeros((B, S, DIM), dtype=np.float32)
    for c in range(8):
        o = res.results[c]["outc"].astype(np.float32)      # (8,4,128,512)
        outT = o.transpose(0, 2, 1, 3).reshape(DIM, S)
        out[c // 4] += outT.T
    return out


# revision 15
# speedup vs baseline: 1.0025x; 1.0025x over previous
"""Block-causal GQA attention for Trainium2, 8 NeuronCores.

Sharding: core = (batch b, GQA group g): 2 batches x 4 kv-groups.
Each core computes its 4 q-heads + 1 kv-head on one batch element in a
transposed layout (head_dim on partitions, tokens on free dim), then a
row-parallel partial out-projection; the host sums the 4 partials per batch.

v3 structure (vs v2 baseline):
  - All DRAM inputs host-packed to SBUF layout (contiguous DMAs); critical
    path (wq, x chunk 0) leads the SP queue, tables on the Act queue,
    phase-A-only data on the Pool queue.
  - One PSUM pool set for the whole kernel (tags A1/A2/pv = 8 banks), no
    phase-boundary barrier.
  - Phase P: k-RMS folded into kt2 (pre-rope scale), q/k norm sqrt+recip
    merged into one scalar + one DVE call via tile-position packing into a
    [66,512] tile, rotate-half via strip DMAs (off the DVE), norm-broadcast
    matmuls deferred to a P2 pass so the PE queue never stalls on the
    sqrt/recip chain.
  - Phase A: exp with immediate scale/bias (k-norm pre-applied), PV pair
    merged into one matmul (2D free AP), 2-entry scores lookahead, lazy
    softmax normalization (unnormalized evacuation releases the PV PSUM
    early; normalize off-chain before the out-projection).
"""
import numpy as np
import ml_dtypes

B, S, DIM = 2, 2048, 1024
H, KVH, HD = 16, 4, 64
EPS = 1e-6
SCALE = HD ** -0.5
PT_TILES = S // 128  # 16
N_CHUNK = 512
N_CHUNKS = S // N_CHUNK  # 4

_BUILD_CACHE = {}
_BLOCKIND = np.zeros((34, 128), np.float32)
_BLOCKIND[0, 0:64] = 1.0
_BLOCKIND[1, 64:128] = 1.0
_BLOCKIND[32, 0:64] = 1.0
_BLOCKIND[33, 64:128] = 1.0


def _analyze_mask(mask):
    """Classify 128x128 tiles: 0=skip, 1=full, 2=mixed. Returns status grid,
    mixed tile stack (transposed to (k,q) layout, 0/1 float32), and index map.
    Index 0 of the stack is always the all-zero tile."""
    T = PT_TILES
    status = np.zeros((T, T), np.int8)
    tiles = [np.zeros((128, 128), np.float32)]
    idx = {}
    m = np.asarray(mask)
    for i in range(T):
        for j in range(T):
            sub = m[i * 128:(i + 1) * 128, j * 128:(j + 1) * 128]
            if not sub.any():
                status[i, j] = 0
            elif sub.all():
                status[i, j] = 1
            else:
                status[i, j] = 2
                idx[(i, j)] = len(tiles)
                tiles.append(np.ascontiguousarray(sub.T).astype(np.float32))
    return status, np.stack(tiles), idx


def _make_schedule(status, idx):
    """Per chunk: list of (ktile j, s0, s1, [(subtile s, mask_tile_index)])
    where [s0*128, s1*128) is the contiguous span of alive q-subtiles and the
    list holds per-subtile multiplies (zero tile for dead-in-span, mixed id
    for partial)."""
    sched = []
    for ci in range(N_CHUNKS):
        qts = list(range(4 * ci, 4 * ci + 4))
        entries = []
        for j in range(PT_TILES):
            st = [status[i, j] for i in qts]
            if not any(st):
                continue
            alive = [s for s in range(4) if st[s] != 0]
            s0, s1 = alive[0], alive[-1] + 1
            mults = []
            for s in range(s0, s1):
                if st[s] == 1:
                    continue
                mults.append((s, 0 if st[s] == 0 else idx[(qts[s], j)]))
            entries.append((j, s0, s1, mults))
        sched.append(entries)
    return sched


def _build(sched_key, sched, n_masks, neg_c, debug=False):
    import concourse.bacc as bacc
    import concourse.mybir as mybir
    import concourse.tile as tile
    from concourse.masks import make_identity

    F32 = mybir.dt.float32
    F32R = mybir.dt.float32r
    BF16 = mybir.dt.bfloat16
    AF = mybir.ActivationFunctionType

    nc = bacc.Bacc("TRN2", target_bir_lowering=False, debug=False)
    xtc = nc.dram_tensor("xtc", (N_CHUNKS, 128, 8, N_CHUNK), BF16,
                         kind="ExternalInput").ap()
    wq = nc.dram_tensor("wq", (128, 8, 256), BF16, kind="ExternalInput").ap()
    wkv = nc.dram_tensor("wkv", (128, 8, 128), BF16, kind="ExternalInput").ap()
    wo = nc.dram_tensor("wo", (128, 2, DIM), BF16, kind="ExternalInput").ap()
    cq = nc.dram_tensor("cq", (128, S), BF16, kind="ExternalInput").ap()
    sqp = nc.dram_tensor("sqp", (128, S), BF16, kind="ExternalInput").ap()
    ck = nc.dram_tensor("ck", (64, S), BF16, kind="ExternalInput").ap()
    skp = nc.dram_tensor("skp", (64, S), BF16, kind="ExternalInput").ap()
    masks = nc.dram_tensor("masks", (n_masks, 128, 128), BF16,
                           kind="ExternalInput").ap()
    blockind_d = nc.dram_tensor("blockind", (34, 128), F32,
                                kind="ExternalInput").ap()
    outc = nc.dram_tensor("outc", (8, N_CHUNKS, 128, N_CHUNK), BF16,
                          kind="ExternalOutput").ap()
    if debug:
        d_t1a0 = nc.dram_tensor("d_t1a0", (128, S), BF16, kind="ExternalOutput").ap()
        d_t1a1 = nc.dram_tensor("d_t1a1", (128, S), BF16, kind="ExternalOutput").ap()
        d_kt2 = nc.dram_tensor("d_kt2", (128, S), BF16, kind="ExternalOutput").ap()
        d_vaug = nc.dram_tensor("d_vaug", (128, PT_TILES, 65), BF16, kind="ExternalOutput").ap()
        d_an = nc.dram_tensor("d_an", (128, N_CHUNK), BF16, kind="ExternalOutput").ap()
        d_rd = nc.dram_tensor("d_rd", (1, 2, N_CHUNK), mybir.dt.float32, kind="ExternalOutput").ap()
        d_au = nc.dram_tensor("d_au", (64, 2, N_CHUNK), BF16, kind="ExternalOutput").ap()

    with tile.TileContext(nc) as tc:
        with tc.tile_pool(name="persist", bufs=1) as pp, \
             tc.tile_pool(name="work", bufs=2) as pw, \
             tc.tile_pool(name="ps", bufs=2, space="PSUM") as ps, \
             tc.tile_pool(name="psv", bufs=1, space="PSUM") as psv:

            # --- persistent tiles / initial DMAs --------------------------
            # critical path first on SP queue: wq then x chunk 0
            wq_sb = pp.tile([128, 8, 256], BF16)
            nc.sync.dma_start(out=wq_sb[:, 0:4, :], in_=wq[:, 0:4, :])
            # Act queue: x chunk 0 second half first, then weights/tables
            xt0 = None  # placeholder; filled in blob_xt(0)
            wkv_sb = pp.tile([128, 8, 128], BF16)
            cq_sb = pp.tile([128, S], BF16)
            sqp_sb = pp.tile([128, S], BF16)
            ck_sb = pp.tile([64, S], BF16)
            skp_sb = pp.tile([64, S], BF16)
            # Pool queue: phase-A-only data
            blockind = pp.tile([34, 128], F32)
            nc.gpsimd.dma_start(out=blockind, in_=blockind_d)
            masks_sb = pp.tile([128, n_masks, 128], BF16)
            nc.gpsimd.dma_start(out=masks_sb,
                                in_=masks.rearrange("n k q -> k n q"))
            wo_sb = pp.tile([128, 2, DIM], BF16)
            nc.gpsimd.dma_start(out=wo_sb, in_=wo)

            # persistent compute state
            t1s = pp.tile([128, 2, S], BF16)     # roped q, un-normalized
            t1a = [pp.tile([128, S], BF16, tag=f"t1a{m}", name=f"t1a{m}")
                   for m in range(2)]            # roped+normalized q per m
            kt2 = pp.tile([128, S], BF16)        # normalized roped k (dup)
            vst = pp.tile([64, S], BF16)         # v staging (d on part)
            v_aug = pp.tile([128, PT_TILES, 65], BF16)
            nrq = pp.tile([66, N_CHUNKS, N_CHUNK], F32)   # 1/rms: q m0@0:2, m1@32:34, k@64:65
            rkb = pp.tile([64, N_CHUNKS, N_CHUNK], F32)   # k 1/rms bcast 64 part

            ones1 = pp.tile([128, 1], BF16)
            nc.vector.memset(ones1, 1.0)
            nc.vector.tensor_copy(v_aug[:, :, 64:65],
                                  ones1[:].broadcast_to([128, PT_TILES, 1]))
            onesq = pp.tile([128, 2], BF16)
            nc.vector.memset(onesq, 0.0)
            nc.vector.memset(onesq[0:64, 0:1], 1.0)
            nc.vector.memset(onesq[64:128, 1:2], 1.0)
            onesk1 = pp.tile([64, 1], BF16)
            nc.vector.memset(onesk1, 1.0)
            ident_bf = pp.tile([64, 64], BF16)
            make_identity(nc, ident_bf[:])
            eps66 = pp.tile([66, 1], F32)
            nc.vector.memset(eps66, EPS)
            warm = pp.tile([1, 1], F32)
            nc.scalar.activation(out=warm[:], in_=eps66[0:1, 0:1],
                                 func=AF.Log)

            # ============ interleaved P/A pipeline =======================
            # P work for chunk cj+1 is emitted as "blobs" inside the
            # attention groups of chunk cj; one activation table
            # (natural_log_exp) serves the whole kernel: rsqrt = exp(-.5 ln).
            st8 = {}

            def blob_xt(cj):
                t = px.tile([128, 8, N_CHUNK], BF16, tag="xt", name=f"xt{cj}")
                if cj == 0:
                    nc.sync.dma_start(out=t[:, 0:4, :], in_=xtc[0, :, 0:4, :])
                    nc.scalar.dma_start(out=t[:, 4:8, :],
                                        in_=xtc[0, :, 4:8, :])
                    # rest of the Act-queue loads after the critical pieces
                    nc.scalar.dma_start(out=wq_sb[:, 4:8, :],
                                        in_=wq[:, 4:8, :])
                    nc.scalar.dma_start(out=wkv_sb, in_=wkv)
                    nc.scalar.dma_start(out=cq_sb, in_=cq)
                    nc.scalar.dma_start(out=sqp_sb, in_=sqp)
                    nc.scalar.dma_start(out=ck_sb, in_=ck)
                    nc.scalar.dma_start(out=skp_sb, in_=skp)
                else:
                    nc.sync.dma_start(out=t, in_=xtc[cj])
                st8[("xt", cj)] = t

            def blob_q(cj, m):
                xt = st8[("xt", cj)]
                if ("sqq", cj) not in st8:
                    st8[("sqq", cj)] = pw.tile([128, 2, N_CHUNK], BF16,
                                               tag="sqq", name=f"sqq{cj}")
                    st8[("qtr", cj)] = pw.tile([128, 2, N_CHUNK], BF16,
                                               tag="qtr", name=f"qtr{cj}")
                sqq, qtr = st8[("sqq", cj)], st8[("qtr", cj)]
                qp = ps.tile([128, N_CHUNK], F32, tag="A1", name=f"qps{cj}_{m}")
                for k in range(8):
                    nc.tensor.matmul(qp[:], wq_sb[:, k, m * 128:(m + 1) * 128],
                                     xt[:, k, :], start=(k == 0), stop=(k == 7))
                nc.scalar.activation(out=sqq[:, m, :], in_=qp[:],
                                     func=AF.Square)
                nc.scalar.copy(out=qtr[:, m, :], in_=qp[:])

            def blob_kv(cj):
                xt = st8[("xt", cj)]
                off = cj * N_CHUNK
                kv_ps = ps.tile([128, N_CHUNK], F32, tag="A2", name=f"kvps{cj}")
                for k in range(8):
                    nc.tensor.matmul(kv_ps[:], wkv_sb[:, k, :], xt[:, k, :],
                                     start=(k == 0), stop=(k == 7))
                sqk = pw.tile([64, N_CHUNK], BF16, tag="sqk", name=f"sqk{cj}")
                nc.scalar.activation(out=sqk[:], in_=kv_ps[0:64, :],
                                     func=AF.Square)
                ktr_raw = pw.tile([64, N_CHUNK], BF16, tag="ktraw",
                                  name=f"ktraw{cj}")
                nc.vector.tensor_copy(ktr_raw[:], kv_ps[0:64, :])
                nc.vector.tensor_copy(vst[:, off:off + N_CHUNK],
                                      kv_ps[64:128, :])
                st8[("sqk", cj)] = sqk
                st8[("ktraw", cj)] = ktr_raw

            def blob_norms(cj):
                sqq, sqk = st8[("sqq", cj)], st8[("sqk", cj)]
                nrm = ps.tile([66, N_CHUNK], F32, tag="A1", name=f"nrm{cj}")
                nc.tensor.matmul(nrm[0:2, :], onesq[:], sqq[:, 0, :],
                                 start=True, stop=True)
                nc.tensor.matmul(nrm[32:34, :], onesq[:], sqq[:, 1, :],
                                 start=True, stop=True)
                nc.tensor.matmul(nrm[64:65, :], onesk1[:], sqk[:],
                                 start=True, stop=True)
                nsb = pw.tile([66, N_CHUNK], F32, tag="nsb", name=f"nsb{cj}")
                nc.scalar.activation(out=nsb[:], in_=nrm[:], func=AF.Ln,
                                     bias=eps66[:], scale=1.0 / HD)
                nc.scalar.activation(out=nrq[:, cj, :], in_=nsb[:],
                                     func=AF.Exp, scale=-0.5)
                rk0 = pw.tile([1, N_CHUNK], BF16, tag="rk0", name=f"rk0{cj}")
                nc.vector.tensor_copy(rk0[:], nrq[64:65, cj, :])
                rkb = pw.tile([64, N_CHUNK], BF16, tag="rkb", name=f"rkb{cj}")
                nc.gpsimd.partition_broadcast(rkb[:], rk0[:], channels=64)
                st8[("rkb", cj)] = rkb

            def blob_ktail(cj):
                off = cj * N_CHUNK
                ktr_raw, rkb = st8[("ktraw", cj)], st8[("rkb", cj)]
                ktr = pw.tile([64, N_CHUNK], BF16, tag="ktr", name=f"ktr{cj}")
                nc.vector.tensor_mul(ktr[:], ktr_raw[:], rkb[:])
                uk = pw.tile([64, N_CHUNK], BF16, tag="uk", name=f"uk{cj}")
                nc.vector.tensor_mul(uk[:], ktr[:], skp_sb[:, off:off + N_CHUNK])
                tk = pw.tile([64, N_CHUNK], BF16, tag="tk", name=f"tk{cj}")
                nc.vector.tensor_mul(tk[:], ktr[:], ck_sb[:, off:off + N_CHUNK])
                uksw = pw.tile([64, N_CHUNK], BF16, tag="uksw", name=f"uksw{cj}")
                nc.sync.dma_start(out=uksw[0:32, :], in_=uk[32:64, :])
                nc.sync.dma_start(out=uksw[32:64, :], in_=uk[0:32, :])
                # q rope
                qtr = st8[("qtr", cj)]
                uq = pw.tile([128, 2, N_CHUNK], BF16, tag="uq", name=f"uq{cj}")
                tq = pw.tile([128, 2, N_CHUNK], BF16, tag="tq", name=f"tq{cj}")
                for m in range(2):
                    nc.vector.tensor_mul(uq[:, m, :], qtr[:, m, :],
                                         sqp_sb[:, off:off + N_CHUNK])
                    nc.vector.tensor_mul(tq[:, m, :], qtr[:, m, :],
                                         cq_sb[:, off:off + N_CHUNK])
                usw = pw.tile([128, 2, N_CHUNK], BF16, tag="usw",
                              name=f"usw{cj}")
                for blk, srcp in enumerate((32, 0, 96, 64)):
                    nc.sync.dma_start(
                        out=usw[blk * 32:(blk + 1) * 32, :, :],
                        in_=uq[srcp:srcp + 32, :, :])
                nc.vector.tensor_add(kt2[0:64, off:off + N_CHUNK], tk[:],
                                     uksw[:])
                nc.sync.dma_start(out=kt2[64:128, off:off + N_CHUNK],
                                  in_=kt2[0:64, off:off + N_CHUNK])
                nc.vector.tensor_add(t1s[:, :, off:off + N_CHUNK], tq[:],
                                     usw[:])

            def blob_p2a(cj):
                off = cj * N_CHUNK
                for m in range(2):
                    o = 32 * m
                    rep = ps.tile([128, N_CHUNK], F32, tag="A1",
                                  name=f"rep{cj}_{m}")
                    nc.tensor.matmul(rep[:], blockind[o:o + 2, :],
                                     nrq[o:o + 2, cj, :],
                                     start=True, stop=True)
                    nc.vector.tensor_mul(t1a[m][:, off:off + N_CHUNK],
                                         t1s[:, m, off:off + N_CHUNK], rep[:])

            def blob_p2b(cj):
                for t in range(4):
                    j = 4 * cj + t
                    tr_ps = ps.tile([128, 64], BF16, tag="A1",
                                    name=f"tr{cj}_{t}")
                    nc.tensor.transpose(tr_ps[:],
                                        vst[:, j * 128:(j + 1) * 128],
                                        ident_bf[:])
                    nc.vector.tensor_copy(v_aug[:, j, 0:64], tr_ps[:])

            # ======================= PHASE A =============================
            outproj_pending = []
            attn_done = {}

            def issue_outproj_one():
                ci_, mo = outproj_pending.pop(0)
                a0, a1 = attn_done[ci_]
                o_ps = ps.tile([128, N_CHUNK], F32, tag="A1",
                               name=f"ops{ci_}_{mo}")
                nc.tensor.matmul(o_ps[:], wo_sb[:, 0, mo * 128:(mo + 1) * 128],
                                 a0[:], start=True, stop=False)
                nc.tensor.matmul(o_ps[:], wo_sb[:, 1, mo * 128:(mo + 1) * 128],
                                 a1[:], start=False, stop=True)
                o_sb = pw.tile([128, N_CHUNK], BF16, tag="osb", bufs=3,
                               name=f"osb{ci_}_{mo}")
                nc.vector.tensor_copy(o_sb[:], o_ps[:])
                nc.gpsimd.dma_start(out=outc[mo, ci_], in_=o_sb[:])

            def phase2(m, ci, blobs):
                off = ci * N_CHUNK
                entries = sched[ci]
                n_ent = len(entries)
                # blob k fires after entry floor((k+1)*n/(nb+1))
                fire = {}
                for k in range(len(blobs)):
                    e = min(n_ent - 1, (k + 1) * n_ent // (len(blobs) + 1))
                    fire.setdefault(e, []).append(blobs[k])
                pv = psv.tile([65, 2, N_CHUNK], F32, tag="pv",
                              name=f"pv{m}_{ci}")
                hist = []

                def pv_mm(e_idx):
                    a, b_, j, pt = hist[e_idx]
                    for hh in range(2):
                        nc.tensor.matmul(pv[:, hh, a:b_], v_aug[:, j, :],
                                         pt[:, hh, a:b_],
                                         start=(e_idx == 0),
                                         stop=(e_idx == n_ent - 1))

                for idx_e, (j, s0, s1, mults) in enumerate(entries):
                    koff = j * 128
                    a, b_ = s0 * 128, s1 * 128
                    st = ps.tile([128, 2, N_CHUNK], F32, tag="A2",
                                 name=f"st{m}_{ci}_{j}")
                    nc.tensor.matmul(
                        st[:, 0, a:b_],
                        kt2[0:64, koff:koff + 128],
                        t1a[m][0:64, off + a:off + b_],
                        start=True, stop=True)
                    nc.tensor.matmul(
                        st[:, 1, a:b_],
                        kt2[64:128, koff:koff + 128],
                        t1a[m][64:128, off + a:off + b_],
                        start=True, stop=True, tile_position=(64, 0))
                    pt = pw.tile([128, 2, N_CHUNK], BF16, tag="pt", bufs=4,
                                 name=f"pt{m}_{ci}_{j}")
                    nc.scalar.activation(
                        out=pt[:, :, a:b_], in_=st[:, :, a:b_],
                        func=AF.Exp, bias=neg_c, scale=1.0)
                    for s_, mt in mults:
                        nc.vector.tensor_mul(
                            pt[:, :, s_ * 128:(s_ + 1) * 128],
                            pt[:, :, s_ * 128:(s_ + 1) * 128],
                            masks_sb[:, mt:mt + 1, :].broadcast_to([128, 2, 128]))
                    hist.append((a, b_, j, pt))
                    if idx_e >= 2:
                        pv_mm(idx_e - 2)
                    for fn in fire.get(idx_e, ()):
                        fn()
                    if outproj_pending and 2 * idx_e >= n_ent:
                        issue_outproj_one()
                pv_mm(n_ent - 2)
                pv_mm(n_ent - 1)

                # lazy normalization: evacuate unnormalized (releases pv),
                # normalize off the pv-WAR chain
                au = pw.tile([64, 2, N_CHUNK], BF16, tag="au",
                             name=f"au{m}_{ci}")
                nc.vector.tensor_copy(au[:], pv[0:64, :, :])
                dsb = pw.tile([1, 2, N_CHUNK], F32, tag="dsb",
                              name=f"dsb{m}_{ci}")
                nc.vector.tensor_copy(dsb[:], pv[64:65, :, :])
                rd = pw.tile([1, 2, N_CHUNK], F32, tag="rd",
                             name=f"rd{m}_{ci}")
                nc.vector.reciprocal_approx_fast(out=rd[:], in_=dsb[:])
                rd16 = pw.tile([1, 2, N_CHUNK], BF16, tag="rd16",
                               name=f"rd16{m}_{ci}")
                nc.vector.tensor_copy(rd16[:], rd[:])
                rdb = pw.tile([64, 2, N_CHUNK], BF16, tag="rdb",
                              name=f"rdb{m}_{ci}")
                nc.gpsimd.partition_broadcast(rdb[:], rd16[:], channels=64)
                an = pw.tile([128, N_CHUNK], BF16, tag="an", bufs=4,
                             name=f"an{m}_{ci}")
                for hh in range(2):
                    nc.vector.tensor_mul(
                        an[hh * 64:(hh + 1) * 64, :],
                        au[:, hh, :], rdb[:, hh, :])
                if debug and m == 0 and ci == 0:
                    nc.sync.dma_start(out=d_an, in_=an[:])
                    nc.sync.dma_start(out=d_rd, in_=rd[:])
                    nc.sync.dma_start(out=d_au, in_=au[:])
                return an

            with tc.tile_pool(name="px", bufs=2) as px:
                # chunk 0's P runs up front (v transposes deferred into A)
                blob_xt(0)
                blob_q(0, 0)
                blob_q(0, 1)
                blob_kv(0)
                blob_norms(0)
                blob_ktail(0)
                blob_p2a(0)

                for ci in range(N_CHUNKS):
                    cn = ci + 1
                    m0b = [lambda c=ci: blob_p2b(c)]
                    m1b = []
                    if cn < N_CHUNKS:
                        m0b += [lambda c=cn: blob_xt(c),
                                lambda c=cn: blob_q(c, 0),
                                lambda c=cn: blob_q(c, 1),
                                lambda c=cn: blob_kv(c),
                                lambda c=cn: blob_norms(c)]
                        m1b += [lambda c=cn: blob_ktail(c),
                                lambda c=cn: blob_p2a(c)]
                    a0 = phase2(0, ci, m0b)
                    a1 = phase2(1, ci, m1b)
                    attn_done[ci] = (a0, a1)
                    outproj_pending.extend((ci, mo) for mo in range(8))
                while outproj_pending:
                    issue_outproj_one()
                if debug:
                    nc.sync.dma_start(out=d_t1a0, in_=t1a[0][:])
                    nc.sync.dma_start(out=d_t1a1, in_=t1a[1][:])
                    nc.sync.dma_start(out=d_kt2, in_=kt2[:])
                    nc.sync.dma_start(out=d_vaug, in_=v_aug[:])

    nc.compile()
    return nc


def _get_nc(sched_key, sched, n_masks, neg_c):
    key = (sched_key, n_masks, float(neg_c))
    if key not in _BUILD_CACHE:
        _BUILD_CACHE[key] = _build(sched_key, sched, n_masks, neg_c)
    return _BUILD_CACHE[key]


def kernel(x, Wq, Wkv, Wo, q_norm_w, k_norm_w, rope_cos, rope_sin,
           attention_mask):
    x = np.asarray(x, dtype=np.float32)
    Wq = np.asarray(Wq, dtype=np.float32)
    Wkv = np.asarray(Wkv, dtype=np.float32)
    Wo = np.asarray(Wo, dtype=np.float32)
    qw = np.asarray(q_norm_w, dtype=np.float32)
    kw = np.asarray(k_norm_w, dtype=np.float32)
    cos = np.asarray(rope_cos, dtype=np.float32)
    sin = np.asarray(rope_sin, dtype=np.float32)

    status, mask_tiles, idx = _analyze_mask(attention_mask)
    sched = _make_schedule(status, idx)
    sched_key = status.tobytes()

    # numerically safe exp shift (0 in the normal regime)
    mct_q = max(np.abs(cos).max(), np.abs(sin).max(), 1e-9)
    bound = SCALE * 2.0 * HD * mct_q * mct_q \
        * max(np.abs(qw).max(), 1e-9) * max(np.abs(kw).max(), 1e-9)
    neg_c = -max(0.0, float(bound) - 60.0)

    nc = _get_nc(sched_key, sched, mask_tiles.shape[0], neg_c)

    # host-folded rope tables (transposed layout, head-dim on partitions)
    half = HD // 2
    swap = np.concatenate([np.arange(half, HD), np.arange(0, half)])
    sgn = np.concatenate([-np.ones(half, np.float32), np.ones(half, np.float32)])
    BF = ml_dtypes.bfloat16
    # cq[e,t] = qw[e]*SCALE*cos[t,e];  sqp[e,t] = sgn[swap[e]]*qw[e]*SCALE*sin[t,swap[e]]
    cosq_h = (cos.T * (qw * SCALE)[:, None]).astype(np.float32)        # (64,S)
    sinq_h = (sin.T[swap, :] * (sgn[swap] * qw * SCALE)[:, None]).astype(np.float32)
    cosk_h = (cos.T * kw[:, None]).astype(np.float32)
    sink_h = (sin.T[swap, :] * (sgn[swap] * kw)[:, None]).astype(np.float32)
    cq_b = np.ascontiguousarray(np.concatenate([cosq_h, cosq_h], 0)).astype(BF)
    sqp_b = np.ascontiguousarray(np.concatenate([sinq_h, sinq_h], 0)).astype(BF)
    ck_b = np.ascontiguousarray(cosk_h).astype(BF)
    skp_b = np.ascontiguousarray(sink_h).astype(BF)
    masks_b = mask_tiles.astype(BF)

    # x chunks in SBUF layout: (chunk, 128 part, 8 k, 512 n)
    xtc_b = []
    for b in range(B):
        xT = np.ascontiguousarray(x[b].T)                  # (DIM, S)
        v = xT.reshape(8, 128, N_CHUNKS, N_CHUNK).transpose(2, 1, 0, 3)
        xtc_b.append(np.ascontiguousarray(v).astype(BF))

    in_maps = []
    for c in range(8):
        b, g = c // 4, c % 4
        wq_s = Wq[:, g * 256:(g + 1) * 256]                # (1024, 256)
        wq_p = np.ascontiguousarray(
            wq_s.reshape(8, 128, 256).transpose(1, 0, 2)).astype(BF)
        wkv_s = np.concatenate(
            [Wkv[:, g * HD:(g + 1) * HD],
             Wkv[:, KVH * HD + g * HD: KVH * HD + (g + 1) * HD]], axis=1)
        wkv_p = np.ascontiguousarray(
            wkv_s.reshape(8, 128, 128).transpose(1, 0, 2)).astype(BF)
        wo_s = Wo[g * 256:(g + 1) * 256, :]                # (256, 1024)
        wo_p = np.ascontiguousarray(
            wo_s.reshape(2, 128, DIM).transpose(1, 0, 2)).astype(BF)
        im = {
            "xtc": xtc_b[b],
            "wq": wq_p,
            "wkv": wkv_p,
            "wo": wo_p,
            "cq": cq_b, "sqp": sqp_b,
            "ck": ck_b, "skp": skp_b,
            "masks": masks_b,
            "blockind": _BLOCKIND,
        }
        in_maps.append(im)

    from concourse.bass_utils import run_bass_kernel_spmd
    res = run_bass_kernel_spmd(nc, in_maps, core_ids=list(range(8)), trace=False)

    out = np.zeros((B, S, DIM), dtype=np.float32)
    for c in range(8):
        o = res.results[c]["outc"].astype(np.float32)      # (8,4,128,512)
        outT = o.transpose(0, 2, 1, 3).reshape(DIM, S)
        out[c // 4] += outT.T
    return out


# revision 16
# speedup vs baseline: 1.0211x; 1.0185x over previous
"""Block-causal GQA attention for Trainium2, 8 NeuronCores.

Sharding: core = (batch b, GQA group g): 2 batches x 4 kv-groups.
Each core computes its 4 q-heads + 1 kv-head on one batch element in a
transposed layout (head_dim on partitions, tokens on free dim), then a
row-parallel partial out-projection; the host sums the 4 partials per batch.

v3 structure (vs v2 baseline):
  - All DRAM inputs host-packed to SBUF layout (contiguous DMAs); critical
    path (wq, x chunk 0) leads the SP queue, tables on the Act queue,
    phase-A-only data on the Pool queue.
  - One PSUM pool set for the whole kernel (tags A1/A2/pv = 8 banks), no
    phase-boundary barrier.
  - Phase P: k-RMS folded into kt2 (pre-rope scale), q/k norm sqrt+recip
    merged into one scalar + one DVE call via tile-position packing into a
    [66,512] tile, rotate-half via strip DMAs (off the DVE), norm-broadcast
    matmuls deferred to a P2 pass so the PE queue never stalls on the
    sqrt/recip chain.
  - Phase A: exp with immediate scale/bias (k-norm pre-applied), PV pair
    merged into one matmul (2D free AP), 2-entry scores lookahead, lazy
    softmax normalization (unnormalized evacuation releases the PV PSUM
    early; normalize off-chain before the out-projection).
"""
import numpy as np
import ml_dtypes

B, S, DIM = 2, 2048, 1024
H, KVH, HD = 16, 4, 64
EPS = 1e-6
SCALE = HD ** -0.5
PT_TILES = S // 128  # 16
N_CHUNK = 512
N_CHUNKS = S // N_CHUNK  # 4

_BUILD_CACHE = {}
_BLOCKIND = np.zeros((34, 128), np.float32)
_BLOCKIND[0, 0:64] = 1.0
_BLOCKIND[1, 64:128] = 1.0
_BLOCKIND[32, 0:64] = 1.0
_BLOCKIND[33, 64:128] = 1.0


def _analyze_mask(mask):
    """Classify 128x128 tiles: 0=skip, 1=full, 2=mixed. Returns status grid,
    mixed tile stack (transposed to (k,q) layout, 0/1 float32), and index map.
    Index 0 of the stack is always the all-zero tile."""
    T = PT_TILES
    status = np.zeros((T, T), np.int8)
    tiles = [np.zeros((128, 128), np.float32)]
    idx = {}
    m = np.asarray(mask)
    for i in range(T):
        for j in range(T):
            sub = m[i * 128:(i + 1) * 128, j * 128:(j + 1) * 128]
            if not sub.any():
                status[i, j] = 0
            elif sub.all():
                status[i, j] = 1
            else:
                status[i, j] = 2
                idx[(i, j)] = len(tiles)
                tiles.append(np.ascontiguousarray(sub.T).astype(np.float32))
    return status, np.stack(tiles), idx


def _make_schedule(status, idx):
    """Per chunk: list of (ktile j, s0, s1, [(subtile s, mask_tile_index)])
    where [s0*128, s1*128) is the contiguous span of alive q-subtiles and the
    list holds per-subtile multiplies (zero tile for dead-in-span, mixed id
    for partial)."""
    sched = []
    for ci in range(N_CHUNKS):
        qts = list(range(4 * ci, 4 * ci + 4))
        entries = []
        for j in range(PT_TILES):
            st = [status[i, j] for i in qts]
            if not any(st):
                continue
            alive = [s for s in range(4) if st[s] != 0]
            s0, s1 = alive[0], alive[-1] + 1
            mults = []
            for s in range(s0, s1):
                if st[s] == 1:
                    continue
                mults.append((s, 0 if st[s] == 0 else idx[(qts[s], j)]))
            entries.append((j, s0, s1, mults))
        sched.append(entries)
    return sched


def _build(sched_key, sched, n_masks, neg_c, debug=False):
    import concourse.bacc as bacc
    import concourse.mybir as mybir
    import concourse.tile as tile
    from concourse.masks import make_identity

    F32 = mybir.dt.float32
    F32R = mybir.dt.float32r
    BF16 = mybir.dt.bfloat16
    AF = mybir.ActivationFunctionType

    nc = bacc.Bacc("TRN2", target_bir_lowering=False, debug=False)

    # All scalar activations here (Exp/Ln/Square/Copy) live together in the
    # 'natural_log_exp_and_others' table set, but the default first-match
    # policy alternates between 'exp_and_others' and 'natural_log', inserting
    # a ~1.3us ACT_TABLE_LOAD around every norm. Mask the other sets (keeping
    # positional ids intact for walrus) so one load serves the whole kernel.
    import types as _types
    import bass_rust as _bass_rust
    from concourse.hw_specs import get_activation_tables as _gat

    def _one_table_pass(self):
        items = list(_gat(self.m.arch).items())
        keep = "natural_log_exp_and_others"
        tables = [(n, (s if n == keep else set())) for n, s in items]
        _bass_rust.insert_act_table_loads(self, tables)

    nc.insert_act_table_loads = _types.MethodType(_one_table_pass, nc)
    xtc = nc.dram_tensor("xtc", (N_CHUNKS, 128, 8, N_CHUNK), BF16,
                         kind="ExternalInput").ap()
    wq = nc.dram_tensor("wq", (128, 8, 256), BF16, kind="ExternalInput").ap()
    wkv = nc.dram_tensor("wkv", (128, 8, 128), BF16, kind="ExternalInput").ap()
    wo = nc.dram_tensor("wo", (128, 2, DIM), BF16, kind="ExternalInput").ap()
    cq = nc.dram_tensor("cq", (128, S), BF16, kind="ExternalInput").ap()
    sqp = nc.dram_tensor("sqp", (128, S), BF16, kind="ExternalInput").ap()
    ck = nc.dram_tensor("ck", (64, S), BF16, kind="ExternalInput").ap()
    skp = nc.dram_tensor("skp", (64, S), BF16, kind="ExternalInput").ap()
    masks = nc.dram_tensor("masks", (n_masks, 128, 128), BF16,
                           kind="ExternalInput").ap()
    blockind_d = nc.dram_tensor("blockind", (34, 128), F32,
                                kind="ExternalInput").ap()
    outc = nc.dram_tensor("outc", (8, N_CHUNKS, 128, N_CHUNK), BF16,
                          kind="ExternalOutput").ap()
    if debug:
        d_t1a0 = nc.dram_tensor("d_t1a0", (128, S), BF16, kind="ExternalOutput").ap()
        d_t1a1 = nc.dram_tensor("d_t1a1", (128, S), BF16, kind="ExternalOutput").ap()
        d_kt2 = nc.dram_tensor("d_kt2", (128, S), BF16, kind="ExternalOutput").ap()
        d_vaug = nc.dram_tensor("d_vaug", (128, PT_TILES, 65), BF16, kind="ExternalOutput").ap()
        d_an = nc.dram_tensor("d_an", (128, N_CHUNK), BF16, kind="ExternalOutput").ap()
        d_rd = nc.dram_tensor("d_rd", (1, 2, N_CHUNK), mybir.dt.float32, kind="ExternalOutput").ap()
        d_au = nc.dram_tensor("d_au", (64, 2, N_CHUNK), BF16, kind="ExternalOutput").ap()

    with tile.TileContext(nc) as tc:
        with tc.tile_pool(name="persist", bufs=1) as pp, \
             tc.tile_pool(name="work", bufs=2) as pw, \
             tc.tile_pool(name="ps", bufs=2, space="PSUM") as ps, \
             tc.tile_pool(name="psv", bufs=1, space="PSUM") as psv:

            # --- persistent tiles / initial DMAs --------------------------
            # critical path first on SP queue: wq then x chunk 0
            wq_sb = pp.tile([128, 8, 256], BF16)
            nc.sync.dma_start(out=wq_sb[:, 0:4, :], in_=wq[:, 0:4, :])
            # Act queue: x chunk 0 second half first, then weights/tables
            xt0 = None  # placeholder; filled in blob_xt(0)
            wkv_sb = pp.tile([128, 8, 128], BF16)
            cq_sb = pp.tile([128, S], BF16)
            sqp_sb = pp.tile([128, S], BF16)
            ck_sb = pp.tile([64, S], BF16)
            skp_sb = pp.tile([64, S], BF16)
            # Pool queue: phase-A-only data
            blockind = pp.tile([34, 128], F32)
            nc.gpsimd.dma_start(out=blockind, in_=blockind_d)
            masks_sb = pp.tile([128, n_masks, 128], BF16)
            nc.gpsimd.dma_start(out=masks_sb,
                                in_=masks.rearrange("n k q -> k n q"))
            wo_sb = pp.tile([128, 2, DIM], BF16)
            nc.gpsimd.dma_start(out=wo_sb, in_=wo)

            # persistent compute state
            t1s = pp.tile([128, 2, S], BF16)     # roped q, un-normalized
            t1a = [pp.tile([128, S], BF16, tag=f"t1a{m}", name=f"t1a{m}")
                   for m in range(2)]            # roped+normalized q per m
            kt2 = pp.tile([128, S], BF16)        # normalized roped k (dup)
            vst = pp.tile([64, S], BF16)         # v staging (d on part)
            v_aug = pp.tile([128, PT_TILES, 65], BF16)
            nrq = pp.tile([66, N_CHUNKS, N_CHUNK], F32)   # 1/rms: q m0@0:2, m1@32:34, k@64:65
            rkb = pp.tile([64, N_CHUNKS, N_CHUNK], F32)   # k 1/rms bcast 64 part

            ones1 = pp.tile([128, 1], BF16)
            nc.vector.memset(ones1, 1.0)
            nc.vector.tensor_copy(v_aug[:, :, 64:65],
                                  ones1[:].broadcast_to([128, PT_TILES, 1]))
            onesq = pp.tile([128, 2], BF16)
            nc.vector.memset(onesq, 0.0)
            nc.vector.memset(onesq[0:64, 0:1], 1.0)
            nc.vector.memset(onesq[64:128, 1:2], 1.0)
            onesk1 = pp.tile([64, 1], BF16)
            nc.vector.memset(onesk1, 1.0)
            ident_bf = pp.tile([64, 64], BF16)
            make_identity(nc, ident_bf[:])
            eps66 = pp.tile([66, 1], F32)
            nc.vector.memset(eps66, EPS)
            warm = pp.tile([1, 1], F32)
            nc.scalar.activation(out=warm[:], in_=eps66[0:1, 0:1],
                                 func=AF.Log)

            # ============ interleaved P/A pipeline =======================
            # P work for chunk cj+1 is emitted as "blobs" inside the
            # attention groups of chunk cj; one activation table
            # (natural_log_exp) serves the whole kernel: rsqrt = exp(-.5 ln).
            st8 = {}

            def blob_xt(cj):
                t = px.tile([128, 8, N_CHUNK], BF16, tag="xt", name=f"xt{cj}")
                if cj == 0:
                    nc.sync.dma_start(out=t[:, 0:4, :], in_=xtc[0, :, 0:4, :])
                    nc.scalar.dma_start(out=t[:, 4:8, :],
                                        in_=xtc[0, :, 4:8, :])
                    # rest of the Act-queue loads after the critical pieces
                    nc.scalar.dma_start(out=wq_sb[:, 4:8, :],
                                        in_=wq[:, 4:8, :])
                    nc.scalar.dma_start(out=wkv_sb, in_=wkv)
                    nc.scalar.dma_start(out=cq_sb, in_=cq)
                    nc.scalar.dma_start(out=sqp_sb, in_=sqp)
                    nc.scalar.dma_start(out=ck_sb, in_=ck)
                    nc.scalar.dma_start(out=skp_sb, in_=skp)
                else:
                    nc.sync.dma_start(out=t, in_=xtc[cj])
                st8[("xt", cj)] = t

            def blob_q(cj, m):
                xt = st8[("xt", cj)]
                if ("sqq", cj) not in st8:
                    st8[("sqq", cj)] = pw.tile([128, 2, N_CHUNK], BF16,
                                               tag="sqq", name=f"sqq{cj}")
                    st8[("qtr", cj)] = pw.tile([128, 2, N_CHUNK], BF16,
                                               tag="qtr", name=f"qtr{cj}")
                sqq, qtr = st8[("sqq", cj)], st8[("qtr", cj)]
                qp = ps.tile([128, N_CHUNK], F32, tag="A1", name=f"qps{cj}_{m}")
                for k in range(8):
                    nc.tensor.matmul(qp[:], wq_sb[:, k, m * 128:(m + 1) * 128],
                                     xt[:, k, :], start=(k == 0), stop=(k == 7))
                nc.scalar.activation(out=sqq[:, m, :], in_=qp[:],
                                     func=AF.Square)
                nc.scalar.copy(out=qtr[:, m, :], in_=qp[:])

            def blob_kv(cj):
                xt = st8[("xt", cj)]
                off = cj * N_CHUNK
                kv_ps = ps.tile([128, N_CHUNK], F32, tag="A2", name=f"kvps{cj}")
                for k in range(8):
                    nc.tensor.matmul(kv_ps[:], wkv_sb[:, k, :], xt[:, k, :],
                                     start=(k == 0), stop=(k == 7))
                sqk = pw.tile([64, N_CHUNK], BF16, tag="sqk", name=f"sqk{cj}")
                nc.scalar.activation(out=sqk[:], in_=kv_ps[0:64, :],
                                     func=AF.Square)
                ktr_raw = pw.tile([64, N_CHUNK], BF16, tag="ktraw",
                                  name=f"ktraw{cj}")
                nc.vector.tensor_copy(ktr_raw[:], kv_ps[0:64, :])
                nc.vector.tensor_copy(vst[:, off:off + N_CHUNK],
                                      kv_ps[64:128, :])
                st8[("sqk", cj)] = sqk
                st8[("ktraw", cj)] = ktr_raw

            def blob_norms(cj):
                sqq, sqk = st8[("sqq", cj)], st8[("sqk", cj)]
                nrm = ps.tile([66, N_CHUNK], F32, tag="A1", name=f"nrm{cj}")
                nc.tensor.matmul(nrm[0:2, :], onesq[:], sqq[:, 0, :],
                                 start=True, stop=True)
                nc.tensor.matmul(nrm[32:34, :], onesq[:], sqq[:, 1, :],
                                 start=True, stop=True)
                nc.tensor.matmul(nrm[64:65, :], onesk1[:], sqk[:],
                                 start=True, stop=True)
                nsb = pw.tile([66, N_CHUNK], F32, tag="nsb", name=f"nsb{cj}")
                nc.scalar.activation(out=nsb[:], in_=nrm[:], func=AF.Ln,
                                     bias=eps66[:], scale=1.0 / HD)
                nc.scalar.activation(out=nrq[:, cj, :], in_=nsb[:],
                                     func=AF.Exp, scale=-0.5)
                rk0 = pw.tile([1, N_CHUNK], BF16, tag="rk0", name=f"rk0{cj}")
                nc.vector.tensor_copy(rk0[:], nrq[64:65, cj, :])
                rkb = pw.tile([64, N_CHUNK], BF16, tag="rkb", name=f"rkb{cj}")
                nc.gpsimd.partition_broadcast(rkb[:], rk0[:], channels=64)
                st8[("rkb", cj)] = rkb

            def blob_ktail(cj):
                off = cj * N_CHUNK
                ktr_raw, rkb = st8[("ktraw", cj)], st8[("rkb", cj)]
                ktr = pw.tile([64, N_CHUNK], BF16, tag="ktr", name=f"ktr{cj}")
                nc.vector.tensor_mul(ktr[:], ktr_raw[:], rkb[:])
                uk = pw.tile([64, N_CHUNK], BF16, tag="uk", name=f"uk{cj}")
                nc.vector.tensor_mul(uk[:], ktr[:], skp_sb[:, off:off + N_CHUNK])
                tk = pw.tile([64, N_CHUNK], BF16, tag="tk", name=f"tk{cj}")
                nc.vector.tensor_mul(tk[:], ktr[:], ck_sb[:, off:off + N_CHUNK])
                uksw = pw.tile([64, N_CHUNK], BF16, tag="uksw", name=f"uksw{cj}")
                nc.sync.dma_start(out=uksw[0:32, :], in_=uk[32:64, :])
                nc.sync.dma_start(out=uksw[32:64, :], in_=uk[0:32, :])
                # q rope
                qtr = st8[("qtr", cj)]
                uq = pw.tile([128, 2, N_CHUNK], BF16, tag="uq", name=f"uq{cj}")
                tq = pw.tile([128, 2, N_CHUNK], BF16, tag="tq", name=f"tq{cj}")
                for m in range(2):
                    nc.vector.tensor_mul(uq[:, m, :], qtr[:, m, :],
                                         sqp_sb[:, off:off + N_CHUNK])
                    nc.vector.tensor_mul(tq[:, m, :], qtr[:, m, :],
                                         cq_sb[:, off:off + N_CHUNK])
                usw = pw.tile([128, 2, N_CHUNK], BF16, tag="usw",
                              name=f"usw{cj}")
                for blk, srcp in enumerate((32, 0, 96, 64)):
                    nc.sync.dma_start(
                        out=usw[blk * 32:(blk + 1) * 32, :, :],
                        in_=uq[srcp:srcp + 32, :, :])
                nc.vector.tensor_add(kt2[0:64, off:off + N_CHUNK], tk[:],
                                     uksw[:])
                nc.sync.dma_start(out=kt2[64:128, off:off + N_CHUNK],
                                  in_=kt2[0:64, off:off + N_CHUNK])
                nc.vector.tensor_add(t1s[:, :, off:off + N_CHUNK], tq[:],
                                     usw[:])

            def blob_p2a(cj):
                off = cj * N_CHUNK
                for m in range(2):
                    o = 32 * m
                    rep = ps.tile([128, N_CHUNK], F32, tag="A1",
                                  name=f"rep{cj}_{m}")
                    nc.tensor.matmul(rep[:], blockind[o:o + 2, :],
                                     nrq[o:o + 2, cj, :],
                                     start=True, stop=True)
                    nc.vector.tensor_mul(t1a[m][:, off:off + N_CHUNK],
                                         t1s[:, m, off:off + N_CHUNK], rep[:])

            def blob_p2b(cj):
                for t in range(4):
                    j = 4 * cj + t
                    tr_ps = ps.tile([128, 64], BF16, tag="A1",
                                    name=f"tr{cj}_{t}")
                    nc.tensor.transpose(tr_ps[:],
                                        vst[:, j * 128:(j + 1) * 128],
                                        ident_bf[:])
                    nc.vector.tensor_copy(v_aug[:, j, 0:64], tr_ps[:])

            # ======================= PHASE A =============================
            outproj_pending = []
            attn_done = {}

            def issue_outproj_one():
                ci_, mo = outproj_pending.pop(0)
                a0, a1 = attn_done[ci_]
                o_ps = ps.tile([128, N_CHUNK], F32, tag="A1",
                               name=f"ops{ci_}_{mo}")
                nc.tensor.matmul(o_ps[:], wo_sb[:, 0, mo * 128:(mo + 1) * 128],
                                 a0[:], start=True, stop=False)
                nc.tensor.matmul(o_ps[:], wo_sb[:, 1, mo * 128:(mo + 1) * 128],
                                 a1[:], start=False, stop=True)
                o_sb = pw.tile([128, N_CHUNK], BF16, tag="osb", bufs=3,
                               name=f"osb{ci_}_{mo}")
                nc.vector.tensor_copy(o_sb[:], o_ps[:])
                nc.gpsimd.dma_start(out=outc[mo, ci_], in_=o_sb[:])

            def phase2(m, ci, blobs):
                off = ci * N_CHUNK
                entries = sched[ci]
                n_ent = len(entries)
                # blob k fires after entry floor((k+1)*n/(nb+1))
                fire = {}
                for k in range(len(blobs)):
                    e = min(n_ent - 1, (k + 1) * n_ent // (len(blobs) + 1))
                    fire.setdefault(e, []).append(blobs[k])
                pv = psv.tile([65, 2, N_CHUNK], F32, tag="pv",
                              name=f"pv{m}_{ci}")
                hist = []

                def pv_mm(e_idx):
                    a, b_, j, pt = hist[e_idx]
                    for hh in range(2):
                        nc.tensor.matmul(pv[:, hh, a:b_], v_aug[:, j, :],
                                         pt[:, hh, a:b_],
                                         start=(e_idx == 0),
                                         stop=(e_idx == n_ent - 1))

                for idx_e, (j, s0, s1, mults) in enumerate(entries):
                    koff = j * 128
                    a, b_ = s0 * 128, s1 * 128
                    st = ps.tile([128, 2, N_CHUNK], F32, tag="A2",
                                 name=f"st{m}_{ci}_{j}")
                    nc.tensor.matmul(
                        st[:, 0, a:b_],
                        kt2[0:64, koff:koff + 128],
                        t1a[m][0:64, off + a:off + b_],
                        start=True, stop=True)
                    nc.tensor.matmul(
                        st[:, 1, a:b_],
                        kt2[64:128, koff:koff + 128],
                        t1a[m][64:128, off + a:off + b_],
                        start=True, stop=True, tile_position=(64, 0))
                    pt = pw.tile([128, 2, N_CHUNK], BF16, tag="pt", bufs=4,
                                 name=f"pt{m}_{ci}_{j}")
                    nc.scalar.activation(
                        out=pt[:, :, a:b_], in_=st[:, :, a:b_],
                        func=AF.Exp, bias=neg_c, scale=1.0)
                    for s_, mt in mults:
                        nc.vector.tensor_mul(
                            pt[:, :, s_ * 128:(s_ + 1) * 128],
                            pt[:, :, s_ * 128:(s_ + 1) * 128],
                            masks_sb[:, mt:mt + 1, :].broadcast_to([128, 2, 128]))
                    hist.append((a, b_, j, pt))
                    if idx_e >= 2:
                        pv_mm(idx_e - 2)
                    for fn in fire.get(idx_e, ()):
                        fn()
                    if outproj_pending and 2 * idx_e >= n_ent:
                        issue_outproj_one()
                pv_mm(n_ent - 2)
                pv_mm(n_ent - 1)

                # lazy normalization: evacuate unnormalized (releases pv),
                # normalize off the pv-WAR chain
                au = pw.tile([64, 2, N_CHUNK], BF16, tag="au",
                             name=f"au{m}_{ci}")
                nc.vector.tensor_copy(au[:], pv[0:64, :, :])
                dsb = pw.tile([1, 2, N_CHUNK], F32, tag="dsb",
                              name=f"dsb{m}_{ci}")
                nc.vector.tensor_copy(dsb[:], pv[64:65, :, :])
                rd = pw.tile([1, 2, N_CHUNK], F32, tag="rd",
                             name=f"rd{m}_{ci}")
                nc.vector.reciprocal_approx_fast(out=rd[:], in_=dsb[:])
                rd16 = pw.tile([1, 2, N_CHUNK], BF16, tag="rd16",
                               name=f"rd16{m}_{ci}")
                nc.vector.tensor_copy(rd16[:], rd[:])
                rdb = pw.tile([64, 2, N_CHUNK], BF16, tag="rdb",
                              name=f"rdb{m}_{ci}")
                nc.gpsimd.partition_broadcast(rdb[:], rd16[:], channels=64)
                an = pw.tile([128, N_CHUNK], BF16, tag="an", bufs=4,
                             name=f"an{m}_{ci}")
                for hh in range(2):
                    nc.vector.tensor_mul(
                        an[hh * 64:(hh + 1) * 64, :],
                        au[:, hh, :], rdb[:, hh, :])
                if debug and m == 0 and ci == 0:
                    nc.sync.dma_start(out=d_an, in_=an[:])
                    nc.sync.dma_start(out=d_rd, in_=rd[:])
                    nc.sync.dma_start(out=d_au, in_=au[:])
                return an

            with tc.tile_pool(name="px", bufs=2) as px:
                # chunk 0's P runs up front (v transposes deferred into A)
                blob_xt(0)
                blob_q(0, 0)
                blob_q(0, 1)
                blob_kv(0)
                blob_norms(0)
                blob_ktail(0)
                blob_p2a(0)

                for ci in range(N_CHUNKS):
                    cn = ci + 1
                    m0b = [lambda c=ci: blob_p2b(c)]
                    m1b = []
                    if cn < N_CHUNKS:
                        m0b += [lambda c=cn: blob_xt(c),
                                lambda c=cn: blob_q(c, 0),
                                lambda c=cn: blob_q(c, 1),
                                lambda c=cn: blob_kv(c),
                                lambda c=cn: blob_norms(c)]
                        m1b += [lambda c=cn: blob_ktail(c),
                                lambda c=cn: blob_p2a(c)]
                    a0 = phase2(0, ci, m0b)
                    a1 = phase2(1, ci, m1b)
                    attn_done[ci] = (a0, a1)
                    outproj_pending.extend((ci, mo) for mo in range(8))
                while outproj_pending:
                    issue_outproj_one()
                if debug:
                    nc.sync.dma_start(out=d_t1a0, in_=t1a[0][:])
                    nc.sync.dma_start(out=d_t1a1, in_=t1a[1][:])
                    nc.sync.dma_start(out=d_kt2, in_=kt2[:])
                    nc.sync.dma_start(out=d_vaug, in_=v_aug[:])

    nc.compile()
    return nc


def _get_nc(sched_key, sched, n_masks, neg_c):
    key = (sched_key, n_masks, float(neg_c))
    if key not in _BUILD_CACHE:
        _BUILD_CACHE[key] = _build(sched_key, sched, n_masks, neg_c)
    return _BUILD_CACHE[key]


def kernel(x, Wq, Wkv, Wo, q_norm_w, k_norm_w, rope_cos, rope_sin,
           attention_mask):
    x = np.asarray(x, dtype=np.float32)
    Wq = np.asarray(Wq, dtype=np.float32)
    Wkv = np.asarray(Wkv, dtype=np.float32)
    Wo = np.asarray(Wo, dtype=np.float32)
    qw = np.asarray(q_norm_w, dtype=np.float32)
    kw = np.asarray(k_norm_w, dtype=np.float32)
    cos = np.asarray(rope_cos, dtype=np.float32)
    sin = np.asarray(rope_sin, dtype=np.float32)

    status, mask_tiles, idx = _analyze_mask(attention_mask)
    sched = _make_schedule(status, idx)
    sched_key = status.tobytes()

    # numerically safe exp shift (0 in the normal regime)
    mct_q = max(np.abs(cos).max(), np.abs(sin).max(), 1e-9)
    bound = SCALE * 2.0 * HD * mct_q * mct_q \
        * max(np.abs(qw).max(), 1e-9) * max(np.abs(kw).max(), 1e-9)
    neg_c = -max(0.0, float(bound) - 60.0)

    nc = _get_nc(sched_key, sched, mask_tiles.shape[0], neg_c)

    # host-folded rope tables (transposed layout, head-dim on partitions)
    half = HD // 2
    swap = np.concatenate([np.arange(half, HD), np.arange(0, half)])
    sgn = np.concatenate([-np.ones(half, np.float32), np.ones(half, np.float32)])
    BF = ml_dtypes.bfloat16
    # cq[e,t] = qw[e]*SCALE*cos[t,e];  sqp[e,t] = sgn[swap[e]]*qw[e]*SCALE*sin[t,swap[e]]
    cosq_h = (cos.T * (qw * SCALE)[:, None]).astype(np.float32)        # (64,S)
    sinq_h = (sin.T[swap, :] * (sgn[swap] * qw * SCALE)[:, None]).astype(np.float32)
    cosk_h = (cos.T * kw[:, None]).astype(np.float32)
    sink_h = (sin.T[swap, :] * (sgn[swap] * kw)[:, None]).astype(np.float32)
    cq_b = np.ascontiguousarray(np.concatenate([cosq_h, cosq_h], 0)).astype(BF)
    sqp_b = np.ascontiguousarray(np.concatenate([sinq_h, sinq_h], 0)).astype(BF)
    ck_b = np.ascontiguousarray(cosk_h).astype(BF)
    skp_b = np.ascontiguousarray(sink_h).astype(BF)
    masks_b = mask_tiles.astype(BF)

    # x chunks in SBUF layout: (chunk, 128 part, 8 k, 512 n)
    xtc_b = []
    for b in range(B):
        xT = np.ascontiguousarray(x[b].T)                  # (DIM, S)
        v = xT.reshape(8, 128, N_CHUNKS, N_CHUNK).transpose(2, 1, 0, 3)
        xtc_b.append(np.ascontiguousarray(v).astype(BF))

    in_maps = []
    for c in range(8):
        b, g = c // 4, c % 4
        wq_s = Wq[:, g * 256:(g + 1) * 256]                # (1024, 256)
        wq_p = np.ascontiguousarray(
            wq_s.reshape(8, 128, 256).transpose(1, 0, 2)).astype(BF)
        wkv_s = np.concatenate(
            [Wkv[:, g * HD:(g + 1) * HD],
             Wkv[:, KVH * HD + g * HD: KVH * HD + (g + 1) * HD]], axis=1)
        wkv_p = np.ascontiguousarray(
            wkv_s.reshape(8, 128, 128).transpose(1, 0, 2)).astype(BF)
        wo_s = Wo[g * 256:(g + 1) * 256, :]                # (256, 1024)
        wo_p = np.ascontiguousarray(
            wo_s.reshape(2, 128, DIM).transpose(1, 0, 2)).astype(BF)
        im = {
            "xtc": xtc_b[b],
            "wq": wq_p,
            "wkv": wkv_p,
            "wo": wo_p,
            "cq": cq_b, "sqp": sqp_b,
            "ck": ck_b, "skp": skp_b,
            "masks": masks_b,
            "blockind": _BLOCKIND,
        }
        in_maps.append(im)

    from concourse.bass_utils import run_bass_kernel_spmd
    res = run_bass_kernel_spmd(nc, in_maps, core_ids=list(range(8)), trace=False)

    out = np.zeros((B, S, DIM), dtype=np.float32)
    for c in range(8):
        o = res.results[c]["outc"].astype(np.float32)      # (8,4,128,512)
        outT = o.transpose(0, 2, 1, 3).reshape(DIM, S)
        out[c // 4] += outT.T
    return out


# revision 19
# speedup vs baseline: 1.0538x; 1.0320x over previous
"""Block-causal GQA attention for Trainium2, 8 NeuronCores.

Sharding: core = (batch b, GQA group g): 2 batches x 4 kv-groups.
Each core computes its 4 q-heads + 1 kv-head on one batch element in a
transposed layout (head_dim on partitions, tokens on free dim), then a
row-parallel partial out-projection; the host sums the 4 partials per batch.

v3 structure (vs v2 baseline):
  - All DRAM inputs host-packed to SBUF layout (contiguous DMAs); critical
    path (wq, x chunk 0) leads the SP queue, tables on the Act queue,
    phase-A-only data on the Pool queue.
  - One PSUM pool set for the whole kernel (tags A1/A2/pv = 8 banks), no
    phase-boundary barrier.
  - Phase P: k-RMS folded into kt2 (pre-rope scale), q/k norm sqrt+recip
    merged into one scalar + one DVE call via tile-position packing into a
    [66,512] tile, rotate-half via strip DMAs (off the DVE), norm-broadcast
    matmuls deferred to a P2 pass so the PE queue never stalls on the
    sqrt/recip chain.
  - Phase A: exp with immediate scale/bias (k-norm pre-applied), PV pair
    merged into one matmul (2D free AP), 2-entry scores lookahead, lazy
    softmax normalization (unnormalized evacuation releases the PV PSUM
    early; normalize off-chain before the out-projection).
"""
import numpy as np
import ml_dtypes

B, S, DIM = 2, 2048, 1024
H, KVH, HD = 16, 4, 64
EPS = 1e-6
SCALE = HD ** -0.5
PT_TILES = S // 128  # 16
N_CHUNK = 512
N_CHUNKS = S // N_CHUNK  # 4

_BUILD_CACHE = {}
_BLOCKIND = np.zeros((34, 128), np.float32)
_BLOCKIND[0, 0:64] = 1.0
_BLOCKIND[1, 64:128] = 1.0
_BLOCKIND[32, 0:64] = 1.0
_BLOCKIND[33, 64:128] = 1.0


def _analyze_mask(mask):
    """Classify 128x128 tiles: 0=skip, 1=full, 2=mixed. Returns status grid,
    mixed tile stack (transposed to (k,q) layout, 0/1 float32), and index map.
    Index 0 of the stack is always the all-zero tile."""
    T = PT_TILES
    status = np.zeros((T, T), np.int8)
    tiles = [np.zeros((128, 128), np.float32)]
    idx = {}
    m = np.asarray(mask)
    for i in range(T):
        for j in range(T):
            sub = m[i * 128:(i + 1) * 128, j * 128:(j + 1) * 128]
            if not sub.any():
                status[i, j] = 0
            elif sub.all():
                status[i, j] = 1
            else:
                status[i, j] = 2
                idx[(i, j)] = len(tiles)
                tiles.append(np.ascontiguousarray(sub.T).astype(np.float32))
    return status, np.stack(tiles), idx


def _make_schedule(status, idx):
    """Per chunk: list of (ktile j, s0, s1, [(subtile s, mask_tile_index)])
    where [s0*128, s1*128) is the contiguous span of alive q-subtiles and the
    list holds per-subtile multiplies (zero tile for dead-in-span, mixed id
    for partial)."""
    sched = []
    for ci in range(N_CHUNKS):
        qts = list(range(4 * ci, 4 * ci + 4))
        entries = []
        for j in range(PT_TILES):
            st = [status[i, j] for i in qts]
            if not any(st):
                continue
            alive = [s for s in range(4) if st[s] != 0]
            s0, s1 = alive[0], alive[-1] + 1
            mults = []
            for s in range(s0, s1):
                if st[s] == 1:
                    continue
                mults.append((s, 0 if st[s] == 0 else idx[(qts[s], j)]))
            entries.append((j, s0, s1, mults))
        sched.append(entries)
    return sched


def _build(sched_key, sched, n_masks, neg_c, debug=False):
    import concourse.bacc as bacc
    import concourse.mybir as mybir
    import concourse.tile as tile
    from concourse.masks import make_identity

    F32 = mybir.dt.float32
    F32R = mybir.dt.float32r
    BF16 = mybir.dt.bfloat16
    AF = mybir.ActivationFunctionType

    nc = bacc.Bacc("TRN2", target_bir_lowering=False, debug=False)

    # All scalar activations here (Exp/Ln/Square/Copy) live together in the
    # 'natural_log_exp_and_others' table set, but the default first-match
    # policy alternates between 'exp_and_others' and 'natural_log', inserting
    # a ~1.3us ACT_TABLE_LOAD around every norm. Mask the other sets (keeping
    # positional ids intact for walrus) so one load serves the whole kernel.
    import types as _types
    import bass_rust as _bass_rust
    from concourse.hw_specs import get_activation_tables as _gat

    def _one_table_pass(self):
        items = list(_gat(self.m.arch).items())
        keep = "natural_log_exp_and_others"
        tables = [(n, (s if n == keep else set())) for n, s in items]
        _bass_rust.insert_act_table_loads(self, tables)

    nc.insert_act_table_loads = _types.MethodType(_one_table_pass, nc)
    xtc = nc.dram_tensor("xtc", (N_CHUNKS, 128, 8, N_CHUNK), BF16,
                         kind="ExternalInput").ap()
    wq = nc.dram_tensor("wq", (128, 8, 256), BF16, kind="ExternalInput").ap()
    wkv = nc.dram_tensor("wkv", (128, 8, 128), BF16, kind="ExternalInput").ap()
    wo = nc.dram_tensor("wo", (128, 2, DIM), BF16,
                        kind="ExternalInput").ap()
    cq = nc.dram_tensor("cq", (128, S), BF16, kind="ExternalInput").ap()
    sqp = nc.dram_tensor("sqp", (128, S), BF16, kind="ExternalInput").ap()
    ck = nc.dram_tensor("ck", (64, S), BF16, kind="ExternalInput").ap()
    skp = nc.dram_tensor("skp", (64, S), BF16, kind="ExternalInput").ap()
    masks = nc.dram_tensor("masks", (n_masks, 128, 128), BF16,
                           kind="ExternalInput").ap()
    blockind_d = nc.dram_tensor("blockind", (34, 128), BF16,
                                kind="ExternalInput").ap()
    outc = nc.dram_tensor("outc", (8, N_CHUNKS, 128, N_CHUNK), BF16,
                          kind="ExternalOutput").ap()
    if debug:
        d_t1a0 = nc.dram_tensor("d_t1a0", (128, S), BF16, kind="ExternalOutput").ap()
        d_t1a1 = nc.dram_tensor("d_t1a1", (128, S), BF16, kind="ExternalOutput").ap()
        d_kt2 = nc.dram_tensor("d_kt2", (128, S), BF16, kind="ExternalOutput").ap()
        d_vaug = nc.dram_tensor("d_vaug", (128, PT_TILES, 65), BF16, kind="ExternalOutput").ap()
        d_rd = nc.dram_tensor("d_rd", (1, 2, N_CHUNK), mybir.dt.float32, kind="ExternalOutput").ap()
        d_au = nc.dram_tensor("d_au", (64, 2, N_CHUNK), BF16, kind="ExternalOutput").ap()

    with tile.TileContext(nc) as tc:
        with tc.tile_pool(name="persist", bufs=1) as pp, \
             tc.tile_pool(name="work", bufs=2) as pw, \
             tc.tile_pool(name="ps", bufs=2, space="PSUM") as ps, \
             tc.tile_pool(name="psv", bufs=1, space="PSUM") as psv:

            # --- persistent tiles / initial DMAs --------------------------
            # critical path first on SP queue: wq then x chunk 0
            wq_sb = pp.tile([128, 8, 256], BF16)
            nc.sync.dma_start(out=wq_sb[:, 0:4, :], in_=wq[:, 0:4, :])
            # Act queue: x chunk 0 second half first, then weights/tables
            xt0 = None  # placeholder; filled in blob_xt(0)
            wkv_sb = pp.tile([128, 8, 128], BF16)
            cq_sb = pp.tile([128, S], BF16)
            sqp_sb = pp.tile([128, S], BF16)
            ck_sb = pp.tile([64, S], BF16)
            skp_sb = pp.tile([64, S], BF16)
            # Pool queue: phase-A-only data
            blockind = pp.tile([34, 128], BF16)
            nc.gpsimd.dma_start(out=blockind, in_=blockind_d)
            masks_sb = pp.tile([128, n_masks, 128], BF16)
            nc.gpsimd.dma_start(out=masks_sb,
                                in_=masks.rearrange("n k q -> k n q"))
            wo_sb = pp.tile([128, 2, DIM], BF16)
            nc.gpsimd.dma_start(out=wo_sb, in_=wo)

            # persistent compute state
            t1s = pp.tile([128, 2, S], BF16)     # roped q, un-normalized
            t1a = [pp.tile([128, S], BF16, tag=f"t1a{m}", name=f"t1a{m}")
                   for m in range(2)]            # roped+normalized q per m
            kt2 = pp.tile([128, S], BF16)        # normalized roped k (dup)
            vst = pp.tile([64, S], BF16)         # v staging (d on part)
            v_aug = pp.tile([128, PT_TILES, 65], BF16)
            nrq = pp.tile([66, N_CHUNKS, N_CHUNK], BF16)  # 1/rms: q m0@0:2, m1@32:34, k@64:65
            rkb = pp.tile([64, N_CHUNKS, N_CHUNK], F32)   # k 1/rms bcast 64 part

            ones1 = pp.tile([128, 1], BF16)
            nc.vector.memset(ones1, 1.0)
            nc.vector.tensor_copy(v_aug[:, :, 64:65],
                                  ones1[:].broadcast_to([128, PT_TILES, 1]))
            onesq = pp.tile([128, 2], BF16)
            nc.vector.memset(onesq, 0.0)
            nc.vector.memset(onesq[0:64, 0:1], 1.0)
            nc.vector.memset(onesq[64:128, 1:2], 1.0)
            onesk1 = pp.tile([64, 1], BF16)
            nc.vector.memset(onesk1, 1.0)
            ident_bf = pp.tile([64, 64], BF16)
            make_identity(nc, ident_bf[:])
            eps66 = pp.tile([66, 1], F32)
            nc.vector.memset(eps66, EPS)
            warm = pp.tile([1, 1], F32)
            nc.scalar.activation(out=warm[:], in_=eps66[0:1, 0:1],
                                 func=AF.Log)

            # ============ interleaved P/A pipeline =======================
            # P work for chunk cj+1 is emitted as "blobs" inside the
            # attention groups of chunk cj; one activation table
            # (natural_log_exp) serves the whole kernel: rsqrt = exp(-.5 ln).
            st8 = {}

            def blob_xt(cj):
                t = px.tile([128, 8, N_CHUNK], BF16, tag="xt", name=f"xt{cj}")
                if cj == 0:
                    nc.sync.dma_start(out=t[:, 0:4, :], in_=xtc[0, :, 0:4, :])
                    nc.scalar.dma_start(out=t[:, 4:8, :],
                                        in_=xtc[0, :, 4:8, :])
                    # rest of the Act-queue loads after the critical pieces
                    nc.scalar.dma_start(out=wq_sb[:, 4:8, :],
                                        in_=wq[:, 4:8, :])
                    nc.scalar.dma_start(out=wkv_sb, in_=wkv)
                    nc.scalar.dma_start(out=cq_sb, in_=cq)
                    nc.scalar.dma_start(out=sqp_sb, in_=sqp)
                    nc.scalar.dma_start(out=ck_sb, in_=ck)
                    nc.scalar.dma_start(out=skp_sb, in_=skp)
                else:
                    nc.sync.dma_start(out=t, in_=xtc[cj])
                st8[("xt", cj)] = t

            def blob_q(cj, m):
                xt = st8[("xt", cj)]
                if ("sqq", cj) not in st8:
                    st8[("sqq", cj)] = pw.tile([128, 2, N_CHUNK], BF16,
                                               tag="sqq", name=f"sqq{cj}")
                    st8[("qtr", cj)] = pw.tile([128, 2, N_CHUNK], BF16,
                                               tag="qtr", name=f"qtr{cj}")
                sqq, qtr = st8[("sqq", cj)], st8[("qtr", cj)]
                qp = ps.tile([128, N_CHUNK], F32, tag="A1", name=f"qps{cj}_{m}")
                for k in range(8):
                    nc.tensor.matmul(qp[:], wq_sb[:, k, m * 128:(m + 1) * 128],
                                     xt[:, k, :], start=(k == 0), stop=(k == 7))
                nc.scalar.activation(out=sqq[:, m, :], in_=qp[:],
                                     func=AF.Square)
                nc.scalar.copy(out=qtr[:, m, :], in_=qp[:])

            def blob_kv(cj):
                xt = st8[("xt", cj)]
                off = cj * N_CHUNK
                kv_ps = ps.tile([128, N_CHUNK], F32, tag="A2", name=f"kvps{cj}")
                for k in range(8):
                    nc.tensor.matmul(kv_ps[:], wkv_sb[:, k, :], xt[:, k, :],
                                     start=(k == 0), stop=(k == 7))
                sqk = pw.tile([64, N_CHUNK], BF16, tag="sqk", name=f"sqk{cj}")
                nc.scalar.activation(out=sqk[:], in_=kv_ps[0:64, :],
                                     func=AF.Square)
                ktr_raw = pw.tile([64, N_CHUNK], BF16, tag="ktraw",
                                  name=f"ktraw{cj}")
                nc.vector.tensor_copy(ktr_raw[:], kv_ps[0:64, :])
                nc.vector.tensor_copy(vst[:, off:off + N_CHUNK],
                                      kv_ps[64:128, :])
                st8[("sqk", cj)] = sqk
                st8[("ktraw", cj)] = ktr_raw

            def blob_norms(cj):
                sqq, sqk = st8[("sqq", cj)], st8[("sqk", cj)]
                nrm = ps.tile([66, N_CHUNK], F32, tag="A1", name=f"nrm{cj}")
                nc.tensor.matmul(nrm[0:2, :], onesq[:], sqq[:, 0, :],
                                 start=True, stop=True)
                nc.tensor.matmul(nrm[32:34, :], onesq[:], sqq[:, 1, :],
                                 start=True, stop=True)
                nc.tensor.matmul(nrm[64:65, :], onesk1[:], sqk[:],
                                 start=True, stop=True)
                nsb = pw.tile([66, N_CHUNK], F32, tag="nsb", name=f"nsb{cj}")
                nc.scalar.activation(out=nsb[:], in_=nrm[:], func=AF.Ln,
                                     bias=eps66[:], scale=1.0 / HD)
                nc.scalar.activation(out=nrq[:, cj, :], in_=nsb[:],
                                     func=AF.Exp, scale=-0.5)
                rk0 = pw.tile([1, N_CHUNK], BF16, tag="rk0", name=f"rk0{cj}")
                nc.vector.tensor_copy(rk0[:], nrq[64:65, cj, :])
                rkb = pw.tile([64, N_CHUNK], BF16, tag="rkb", name=f"rkb{cj}")
                nc.gpsimd.partition_broadcast(rkb[:], rk0[:], channels=64)
                st8[("rkb", cj)] = rkb

            def blob_ktail(cj):
                off = cj * N_CHUNK
                ktr_raw, rkb = st8[("ktraw", cj)], st8[("rkb", cj)]
                ktr = pw.tile([64, N_CHUNK], BF16, tag="ktr", name=f"ktr{cj}")
                nc.vector.tensor_mul(ktr[:], ktr_raw[:], rkb[:])
                uk = pw.tile([64, N_CHUNK], BF16, tag="uk", name=f"uk{cj}")
                nc.vector.tensor_mul(uk[:], ktr[:], skp_sb[:, off:off + N_CHUNK])
                tk = pw.tile([64, N_CHUNK], BF16, tag="tk", name=f"tk{cj}")
                nc.vector.tensor_mul(tk[:], ktr[:], ck_sb[:, off:off + N_CHUNK])
                uksw = pw.tile([64, N_CHUNK], BF16, tag="uksw", name=f"uksw{cj}")
                nc.sync.dma_start(out=uksw[0:32, :], in_=uk[32:64, :])
                nc.sync.dma_start(out=uksw[32:64, :], in_=uk[0:32, :])
                # q rope
                qtr = st8[("qtr", cj)]
                uq = pw.tile([128, 2, N_CHUNK], BF16, tag="uq", name=f"uq{cj}")
                tq = pw.tile([128, 2, N_CHUNK], BF16, tag="tq", name=f"tq{cj}")
                for m in range(2):
                    nc.vector.tensor_mul(uq[:, m, :], qtr[:, m, :],
                                         sqp_sb[:, off:off + N_CHUNK])
                    nc.vector.tensor_mul(tq[:, m, :], qtr[:, m, :],
                                         cq_sb[:, off:off + N_CHUNK])
                usw = pw.tile([128, 2, N_CHUNK], BF16, tag="usw",
                              name=f"usw{cj}")
                for blk, srcp in enumerate((32, 0, 96, 64)):
                    nc.sync.dma_start(
                        out=usw[blk * 32:(blk + 1) * 32, :, :],
                        in_=uq[srcp:srcp + 32, :, :])
                nc.vector.tensor_add(kt2[0:64, off:off + N_CHUNK], tk[:],
                                     uksw[:])
                nc.sync.dma_start(out=kt2[64:128, off:off + N_CHUNK],
                                  in_=kt2[0:64, off:off + N_CHUNK])
                nc.vector.tensor_add(t1s[:, :, off:off + N_CHUNK], tq[:],
                                     usw[:])

            def blob_p2a(cj):
                off = cj * N_CHUNK
                for m in range(2):
                    o = 32 * m
                    rep = ps.tile([128, N_CHUNK], F32, tag="A1",
                                  name=f"rep{cj}_{m}")
                    nc.tensor.matmul(rep[:], blockind[o:o + 2, :],
                                     nrq[o:o + 2, cj, :],
                                     start=True, stop=True)
                    nc.vector.tensor_mul(t1a[m][:, off:off + N_CHUNK],
                                         t1s[:, m, off:off + N_CHUNK], rep[:])

            def blob_p2b(cj):
                for t in range(4):
                    j = 4 * cj + t
                    tr_ps = ps.tile([128, 64], BF16, tag="A1",
                                    name=f"tr{cj}_{t}")
                    nc.tensor.transpose(tr_ps[:],
                                        vst[:, j * 128:(j + 1) * 128],
                                        ident_bf[:])
                    nc.vector.tensor_copy(v_aug[:, j, 0:64], tr_ps[:])

            # ======================= PHASE A =============================
            outproj_pending = []
            attn_done = {}

            def issue_outproj_one():
                ci_, mo = outproj_pending.pop(0)
                andr = attn_done[ci_]
                o_ps = ps.tile([128, N_CHUNK], F32, tag="A1",
                               name=f"ops{ci_}_{mo}")
                nc.tensor.matmul(o_ps[:], wo_sb[:, 0, mo * 128:(mo + 1) * 128],
                                 andr[:, 0, :], start=True, stop=False)
                nc.tensor.matmul(o_ps[:], wo_sb[:, 1, mo * 128:(mo + 1) * 128],
                                 andr[:, 1, :], start=False, stop=True)
                o_sb = pw.tile([128, N_CHUNK], BF16, tag="osb", bufs=3,
                               name=f"osb{ci_}_{mo}")
                nc.vector.tensor_copy(o_sb[:], o_ps[:])
                nc.gpsimd.dma_start(out=outc[mo, ci_], in_=o_sb[:])

            def phase2(m, ci, blobs, andr_tile):
                off = ci * N_CHUNK
                entries = sched[ci]
                n_ent = len(entries)
                # blob k fires after entry floor((k+1)*n/(nb+1))
                fire = {}
                for k in range(len(blobs)):
                    e = min(n_ent - 1, (k + 1) * n_ent // (len(blobs) + 1))
                    fire.setdefault(e, []).append(blobs[k])
                pv = psv.tile([65, 2, N_CHUNK], F32, tag="pv",
                              name=f"pv{m}_{ci}")
                hist = []

                def pv_mm(e_idx):
                    a, b_, j, pt = hist[e_idx]
                    for hh in range(2):
                        nc.tensor.matmul(pv[:, hh, a:b_], v_aug[:, j, :],
                                         pt[:, hh, a:b_],
                                         start=(e_idx == 0),
                                         stop=(e_idx == n_ent - 1))

                for idx_e, (j, s0, s1, mults) in enumerate(entries):
                    koff = j * 128
                    a, b_ = s0 * 128, s1 * 128
                    st = ps.tile([128, 2, N_CHUNK], F32, tag="A2",
                                 name=f"st{m}_{ci}_{j}")
                    nc.tensor.matmul(
                        st[:, 0, a:b_],
                        kt2[0:64, koff:koff + 128],
                        t1a[m][0:64, off + a:off + b_],
                        start=True, stop=True)
                    nc.tensor.matmul(
                        st[:, 1, a:b_],
                        kt2[64:128, koff:koff + 128],
                        t1a[m][64:128, off + a:off + b_],
                        start=True, stop=True, tile_position=(64, 0))
                    pt = pw.tile([128, 2, N_CHUNK], BF16, tag="pt", bufs=4,
                                 name=f"pt{m}_{ci}_{j}")
                    nc.scalar.activation(
                        out=pt[:, :, a:b_], in_=st[:, :, a:b_],
                        func=AF.Exp, bias=neg_c, scale=1.0)
                    for s_, mt in mults:
                        nc.vector.tensor_mul(
                            pt[:, :, s_ * 128:(s_ + 1) * 128],
                            pt[:, :, s_ * 128:(s_ + 1) * 128],
                            masks_sb[:, mt:mt + 1, :].broadcast_to([128, 2, 128]))
                    hist.append((a, b_, j, pt))
                    if idx_e >= 2:
                        pv_mm(idx_e - 2)
                    for fn in fire.get(idx_e, ()):
                        fn()
                    if outproj_pending and 2 * idx_e >= n_ent:
                        issue_outproj_one()
                pv_mm(n_ent - 2)
                pv_mm(n_ent - 1)

                # lazy normalization: evacuate unnormalized (releases pv),
                # normalize off the pv-WAR chain
                au = pw.tile([64, 2, N_CHUNK], BF16, tag="au",
                             name=f"au{m}_{ci}")
                nc.vector.tensor_copy(au[:], pv[0:64, :, :])
                dsb = pw.tile([1, 2, N_CHUNK], F32, tag="dsb",
                              name=f"dsb{m}_{ci}")
                nc.vector.tensor_copy(dsb[:], pv[64:65, :, :])
                rd = pw.tile([1, 2, N_CHUNK], F32, tag="rd",
                             name=f"rd{m}_{ci}")
                nc.vector.reciprocal_approx_fast(out=rd[:], in_=dsb[:])
                rd16 = pw.tile([1, 2, N_CHUNK], BF16, tag="rd16",
                               name=f"rd16{m}_{ci}")
                nc.vector.tensor_copy(rd16[:], rd[:])
                rdb = pw.tile([64, 2, N_CHUNK], BF16, tag="rdb",
                              name=f"rdb{m}_{ci}")
                nc.gpsimd.partition_broadcast(rdb[:], rd16[:], channels=64)
                an = andr_tile
                for hh in range(2):
                    nc.vector.tensor_mul(
                        an[hh * 64:(hh + 1) * 64, m, :],
                        au[:, hh, :], rdb[:, hh, :])
                if debug and m == 0 and ci == 0:
                    nc.sync.dma_start(out=d_rd, in_=rd[:])
                    nc.sync.dma_start(out=d_au, in_=au[:])
                return an

            with tc.tile_pool(name="px", bufs=2) as px:
                # chunk 0's P runs up front (v transposes deferred into A)
                blob_xt(0)
                blob_q(0, 0)
                blob_q(0, 1)
                blob_kv(0)
                blob_norms(0)
                blob_ktail(0)
                blob_p2a(0)

                for ci in range(N_CHUNKS):
                    cn = ci + 1
                    m0b = [lambda c=ci: blob_p2b(c)]
                    m1b = []
                    if cn < N_CHUNKS:
                        m0b += [lambda c=cn: blob_xt(c),
                                lambda c=cn: blob_q(c, 0),
                                lambda c=cn: blob_q(c, 1),
                                lambda c=cn: blob_kv(c),
                                lambda c=cn: blob_norms(c)]
                        m1b += [lambda c=cn: blob_ktail(c),
                                lambda c=cn: blob_p2a(c)]
                    andr = pw.tile([128, 2, N_CHUNK], BF16,
                                   tag="andr", bufs=2, name=f"andr{ci}")
                    phase2(0, ci, m0b, andr)
                    phase2(1, ci, m1b, andr)
                    attn_done[ci] = andr
                    outproj_pending.extend((ci, mo) for mo in range(8))
                while outproj_pending:
                    issue_outproj_one()
                if debug:
                    nc.sync.dma_start(out=d_t1a0, in_=t1a[0][:])
                    nc.sync.dma_start(out=d_t1a1, in_=t1a[1][:])
                    nc.sync.dma_start(out=d_kt2, in_=kt2[:])
                    nc.sync.dma_start(out=d_vaug, in_=v_aug[:])

    nc.compile()
    return nc


def _get_nc(sched_key, sched, n_masks, neg_c):
    key = (sched_key, n_masks, float(neg_c))
    if key not in _BUILD_CACHE:
        _BUILD_CACHE[key] = _build(sched_key, sched, n_masks, neg_c)
    return _BUILD_CACHE[key]


def kernel(x, Wq, Wkv, Wo, q_norm_w, k_norm_w, rope_cos, rope_sin,
           attention_mask):
    x = np.asarray(x, dtype=np.float32)
    Wq = np.asarray(Wq, dtype=np.float32)
    Wkv = np.asarray(Wkv, dtype=np.float32)
    Wo = np.asarray(Wo, dtype=np.float32)
    qw = np.asarray(q_norm_w, dtype=np.float32)
    kw = np.asarray(k_norm_w, dtype=np.float32)
    cos = np.asarray(rope_cos, dtype=np.float32)
    sin = np.asarray(rope_sin, dtype=np.float32)

    status, mask_tiles, idx = _analyze_mask(attention_mask)
    sched = _make_schedule(status, idx)
    sched_key = status.tobytes()

    # numerically safe exp shift (0 in the normal regime)
    mct_q = max(np.abs(cos).max(), np.abs(sin).max(), 1e-9)
    bound = SCALE * 2.0 * HD * mct_q * mct_q \
        * max(np.abs(qw).max(), 1e-9) * max(np.abs(kw).max(), 1e-9)
    neg_c = -max(0.0, float(bound) - 60.0)

    nc = _get_nc(sched_key, sched, mask_tiles.shape[0], neg_c)

    # host-folded rope tables (transposed layout, head-dim on partitions)
    half = HD // 2
    swap = np.concatenate([np.arange(half, HD), np.arange(0, half)])
    sgn = np.concatenate([-np.ones(half, np.float32), np.ones(half, np.float32)])
    BF = ml_dtypes.bfloat16
    # cq[e,t] = qw[e]*SCALE*cos[t,e];  sqp[e,t] = sgn[swap[e]]*qw[e]*SCALE*sin[t,swap[e]]
    cosq_h = (cos.T * (qw * SCALE)[:, None]).astype(np.float32)        # (64,S)
    sinq_h = (sin.T[swap, :] * (sgn[swap] * qw * SCALE)[:, None]).astype(np.float32)
    cosk_h = (cos.T * kw[:, None]).astype(np.float32)
    sink_h = (sin.T[swap, :] * (sgn[swap] * kw)[:, None]).astype(np.float32)
    cq_b = np.ascontiguousarray(np.concatenate([cosq_h, cosq_h], 0)).astype(BF)
    sqp_b = np.ascontiguousarray(np.concatenate([sinq_h, sinq_h], 0)).astype(BF)
    ck_b = np.ascontiguousarray(cosk_h).astype(BF)
    skp_b = np.ascontiguousarray(sink_h).astype(BF)
    masks_b = mask_tiles.astype(BF)

    # x chunks in SBUF layout: (chunk, 128 part, 8 k, 512 n)
    xtc_b = []
    for b in range(B):
        xT = np.ascontiguousarray(x[b].T)                  # (DIM, S)
        v = xT.reshape(8, 128, N_CHUNKS, N_CHUNK).transpose(2, 1, 0, 3)
        xtc_b.append(np.ascontiguousarray(v).astype(BF))

    in_maps = []
    for c in range(8):
        b, g = c // 4, c % 4
        wq_s = Wq[:, g * 256:(g + 1) * 256]                # (1024, 256)
        wq_p = np.ascontiguousarray(
            wq_s.reshape(8, 128, 256).transpose(1, 0, 2)).astype(BF)
        wkv_s = np.concatenate(
            [Wkv[:, g * HD:(g + 1) * HD],
             Wkv[:, KVH * HD + g * HD: KVH * HD + (g + 1) * HD]], axis=1)
        wkv_p = np.ascontiguousarray(
            wkv_s.reshape(8, 128, 128).transpose(1, 0, 2)).astype(BF)
        wo_s = Wo[g * 256:(g + 1) * 256, :]                # (256, 1024)
        wo_p = np.ascontiguousarray(
            wo_s.reshape(2, 128, DIM).transpose(1, 0, 2)).astype(BF)
        im = {
            "xtc": xtc_b[b],
            "wq": wq_p,
            "wkv": wkv_p,
            "wo": wo_p,
            "cq": cq_b, "sqp": sqp_b,
            "ck": ck_b, "skp": skp_b,
            "masks": masks_b,
            "blockind": _BLOCKIND.astype(BF),
        }
        in_maps.append(im)

    from concourse.bass_utils import run_bass_kernel_spmd
    res = run_bass_kernel_spmd(nc, in_maps, core_ids=list(range(8)), trace=False)

    out = np.zeros((B, S, DIM), dtype=np.float32)
    for c in range(8):
        o = res.results[c]["outc"].astype(np.float32)      # (8,4,128,512)
        outT = o.transpose(0, 2, 1, 3).reshape(DIM, S)
        out[c // 4] += outT.T
    return out
